# revision 1
# baseline (speedup 1.0000x reference)
"""Bilinear RGGB demosaic (Bayer -> RGB) on 8 Trainium2 NeuronCores.

Layout: batch image n -> core n. Per core, the [2048, 2048] mosaic is
processed in 8 bands of 256 rows; SBUF partition p of a band holds the
row pair (2p, 2p+1) concatenated in the free dim, so every DRAM transfer
is 16 KiB-contiguous per partition. Column-direction stencil taps are
free-dim shifted APs; the two row-direction taps are produced on the
tensor engine as 0.25*(row_{p-1}+row_p) / 0.25*(row_p+row_{p+1}) via
banded 128x128 fp32 matmuls (band-seam rows fixed up with K=2 matmuls
against a 2-row halo tile). VectorE assembles the averaged phases,
ScalarE the pass-through/2x phases, GpSimd the two 0.5x phases.
"""

import sys

sys.path.insert(0, "/opt/trn_rl_repo")

import numpy as np

import concourse.bass as bass
import concourse.tile as tile
from concourse import mybir
from concourse.alu_op_type import AluOpType
from concourse.bass_utils import run_bass_kernel_spmd

F32 = mybir.dt.float32
MM_DT = mybir.dt.float32  # matmul operand dtype (float32r = fast PE mode)
N_CORES = 8
H = 2048
W = 2048
N_BANDS = H // 256  # 128 row-pairs per band


def split_sync_waits(nc, max_waits=1):
    """This walrus build rejects instructions carrying more than
    `max_waits` sync-wait commands. Hoist excess waits onto same-engine
    NoOps inserted immediately before the over-subscribed instruction
    (waiting earlier on the same queue is semantically conservative)."""
    for fn in nc.m.functions:
        for bb in fn.blocks:
            insts = bb.instructions
            i = 0
            while i < len(insts):
                inst = insts[i]
                si = inst.sync_info
                waits = list(si.on_wait) if si and si.on_wait else []
                if len(waits) > max_waits:
                    si.on_wait = waits[:max_waits]
                    excess = waits[max_waits:]
                    for j in range(0, len(excess), max_waits):
                        nop = mybir.InstNoOp(
                            name=nc.get_next_instruction_name(), ins=[], outs=[]
                        )
                        nop.engine = inst.engine
                        nop.sync_info = mybir.SyncInfo(
                            on_wait=excess[j : j + max_waits], on_update=[]
                        )
                        nc.register_instruction(nop)
                        insts.insert(i, nop)
                        i += 1
                i += 1


def const_arrays():
    m1 = 0.25 * (np.eye(128, dtype=np.float32) + np.eye(128, k=1, dtype=np.float32))
    m2 = 0.25 * (np.eye(128, dtype=np.float32) + np.eye(128, k=-1, dtype=np.float32))
    cmm = np.concatenate([m1, m2], axis=1)  # [128, 256]
    cfx = np.zeros((2, 256), dtype=np.float32)
    cfx[0, 0] = 0.25  # fu: Su4[0] += 0.25 * prevO   (halo row 0)
    cfx[1, 128 + 127] = 0.25  # fd: Sd4[127] += 0.25 * nextE (halo row 1)
    return cmm, cfx


def band_plan(npairs):
    """Bands of 128 row-pairs advancing ~126 pairs: each band stores only
    the pair range whose vertical neighbors are in-tile, so no halo or
    seam-fix work is needed. Returns [(start_pair, store_lo, store_hi)]."""
    plan = []
    covered = 0
    while covered < npairs:
        q = 0 if covered == 0 else min(covered - 1, npairs - 128)
        lo = covered - q
        hi = 128 if q + 128 >= npairs else 127
        plan.append((q, lo, hi))
        covered = q + hi
    return plan


def build_program(npairs=H // 2, w=W, repeats=1, variant="full"):
    nc = bass.Bass("TRN2", target_bir_lowering=False, debug=False)
    x = nc.dram_tensor("x", [npairs, 2 * w], F32, kind="ExternalInput").ap()
    cmm = nc.dram_tensor("cmm", [128, 256], F32, kind="ExternalInput").ap()
    out = nc.dram_tensor("out", [3, npairs, 2 * w], F32, kind="ExternalOutput").ap()

    cw = min(512, w)  # matmul free-dim chunk (PSUM bank)
    hw = w // 2
    plan = band_plan(npairs)

    with tile.TileContext(nc) as tc:
        with (
            tc.tile_pool(name="consts", bufs=1) as cpool,
            tc.tile_pool(name="inp", bufs=3) as ipool,
            tc.tile_pool(name="psum", bufs=2, space="PSUM") as ppool,
            tc.tile_pool(name="mids", bufs=2) as mpool,
            tc.tile_pool(name="tmps", bufs=1) as tpool,
            tc.tile_pool(name="outs", bufs=2) as opool,
        ):
            cM = cpool.tile([128, 256], F32)
            nc.sync.dma_start(cM[:], cmm[:])

            def body():
                for q, lo, hi in plan:
                    IN = ipool.tile([128, 2 * w], F32, tag="in")
                    if variant != "nodma":
                        nc.sync.dma_start(IN[:], x[q : q + 128, :])
                    else:
                        nc.gpsimd.memset(IN[:, 0:2], 0.0)

                    E = IN[:, 0:w]
                    O = IN[:, w : 2 * w]

                    # Su4[p] = 0.25*(O[p-1] + O[p]) ; Sd4[p] = 0.25*(E[p] + E[p+1])
                    # PSUM chunked at half-band width, double-buffered.
                    Su4 = mpool.tile([128, w], F32, tag="su_sb")
                    Sd4 = mpool.tile([128, w], F32, tag="sd_sb")
                    pw = min(2 * cw, w)
                    mm = lambda ap: ap.bitcast(MM_DT)
                    for h0 in range(0, w, pw):
                        Su4p = ppool.tile([128, pw], F32, tag="su")
                        Sd4p = ppool.tile([128, pw], F32, tag="sd")
                        for c in range(0, pw, cw):
                            s = slice(h0 + c, h0 + c + cw)
                            sp = slice(c, c + cw)
                            nc.tensor.matmul(
                                Su4p[:, sp], mm(cM[:, 0:128]), mm(O[:, s]),
                                start=True, stop=True,
                            )
                            nc.tensor.matmul(
                                Sd4p[:, sp], mm(cM[:, 128:256]), mm(E[:, s]),
                                start=True, stop=True,
                            )
                        nc.scalar.copy(Su4[:, h0 : h0 + pw], Su4p[:])
                        nc.scalar.copy(Sd4[:, h0 : h0 + pw], Sd4p[:])

                    Rt = opool.tile([128, 2 * w], F32, tag="r")
                    Gt = opool.tile([128, 2 * w], F32, tag="g")
                    Bt = opool.tile([128, 2 * w], F32, tag="b")

                    # ---- R channel ----
                    # even rows, even cols: passthrough E
                    nc.scalar.copy(Rt[:, 0:w:2], E[:, 0:w:2])
                    # even rows, odd cols: 0.5*(E[x-1] + E[x+1])
                    te = tpool.tile([128, hw], F32, tag="te")
                    nc.vector.tensor_add(te[:, 0 : hw - 1], E[:, 0 : w - 2 : 2], E[:, 2:w:2])
                    (nc.scalar.mul if variant == "noGp" else lambda o, i, c: nc.gpsimd.tensor_scalar_mul(o, i, c))(Rt[:, 1 : w - 2 : 2], te[:, 0 : hw - 1], 0.5)
                    nc.vector.tensor_scalar_mul(
                        Rt[:, w - 1 : w], E[:, w - 2 : w - 1], 0.5
                    )
                    # odd rows, even cols: 2*Sd4
                    nc.scalar.mul(Rt[:, w : 2 * w : 2], Sd4[:, 0:w:2], 2.0)
                    # odd rows, odd cols: Sd4[x-1] + Sd4[x+1]
                    nc.vector.tensor_add(
                        Rt[:, w + 1 : 2 * w - 2 : 2], Sd4[:, 0 : w - 2 : 2], Sd4[:, 2:w:2]
                    )
                    nc.vector.tensor_copy(
                        Rt[:, 2 * w - 1 : 2 * w], Sd4[:, w - 2 : w - 1]
                    )

                    # ---- G channel ----
                    # even rows, even cols: 0.25*(E[x-1]+E[x+1]) + Su4[x]
                    tg = tpool.tile([128, hw], F32, tag="tg")
                    nc.vector.tensor_add(tg[:, 0 : hw - 1], E[:, 1 : w - 2 : 2], E[:, 3:w:2])
                    nc.vector.scalar_tensor_tensor(
                        Gt[:, 2 : w - 1 : 2], tg[:, 0 : hw - 1], 0.25,
                        Su4[:, 2 : w - 1 : 2], AluOpType.mult, AluOpType.add,
                    )
                    nc.vector.scalar_tensor_tensor(
                        Gt[:, 0:1], E[:, 1:2], 0.25, Su4[:, 0:1],
                        AluOpType.mult, AluOpType.add,
                    )
                    # even rows, odd cols: passthrough E
                    nc.scalar.copy(Gt[:, 1:w:2], E[:, 1:w:2])
                    # odd rows, even cols: passthrough O
                    nc.scalar.copy(Gt[:, w : 2 * w : 2], O[:, 0:w:2])
                    # odd rows, odd cols: 0.25*(O[x-1]+O[x+1]) + Sd4[x]
                    tg2 = tpool.tile([128, hw], F32, tag="tg2")
                    nc.vector.tensor_add(
                        tg2[:, 0 : hw - 1], O[:, 0 : w - 2 : 2], O[:, 2:w:2]
                    )
                    nc.vector.scalar_tensor_tensor(
                        Gt[:, w + 1 : 2 * w - 2 : 2], tg2[:, 0 : hw - 1], 0.25,
                        Sd4[:, 1 : w - 2 : 2], AluOpType.mult, AluOpType.add,
                    )
                    nc.vector.scalar_tensor_tensor(
                        Gt[:, 2 * w - 1 : 2 * w], O[:, w - 2 : w - 1], 0.25,
                        Sd4[:, w - 1 : w], AluOpType.mult, AluOpType.add,
                    )

                    # ---- B channel ----
                    # even rows, even cols: Su4[x-1] + Su4[x+1]
                    nc.vector.tensor_add(
                        Bt[:, 2 : w - 1 : 2], Su4[:, 1 : w - 2 : 2], Su4[:, 3:w:2]
                    )
                    nc.vector.tensor_copy(Bt[:, 0:1], Su4[:, 1:2])
                    # even rows, odd cols: 2*Su4
                    nc.scalar.mul(Bt[:, 1:w:2], Su4[:, 1:w:2], 2.0)
                    # odd rows, even cols: 0.5*(O[x-1]+O[x+1])
                    tb = tpool.tile([128, hw], F32, tag="tb")
                    nc.vector.tensor_add(tb[:, 0 : hw - 1], O[:, 1 : w - 2 : 2], O[:, 3:w:2])
                    (nc.scalar.mul if variant == "noGp" else lambda o, i, c: nc.gpsimd.tensor_scalar_mul(o, i, c))(
                        Bt[:, w + 2 : 2 * w - 1 : 2], tb[:, 0 : hw - 1], 0.5
                    )
                    nc.vector.tensor_scalar_mul(Bt[:, w : w + 1], O[:, 1:2], 0.5)
                    # odd rows, odd cols: passthrough O
                    nc.scalar.copy(Bt[:, w + 1 : 2 * w : 2], O[:, 1:w:2])

                    if variant != "nodma":
                        for c in range(3):
                            nc.sync.dma_start(
                                out[c, q + lo : q + hi, :], [Rt, Gt, Bt][c][lo:hi, :]
                            )

            if repeats == 1:
                body()
            else:
                with tc.For_i(0, repeats, 1):
                    body()

    split_sync_waits(nc)
    return nc


_CACHE = {}


def _get_program(npairs, w):
    key = (npairs, w)
    if key not in _CACHE:
        _CACHE[key] = build_program(npairs, w)
    return _CACHE[key]


def kernel(x: np.ndarray) -> np.ndarray:
    n, _, h, w = x.shape
    assert (n, h, w) == (N_CORES, H, W), x.shape
    nc = _get_program(H // 2, W)
    cmm, _ = const_arrays()
    in_maps = []
    for i in range(N_CORES):
        img = np.ascontiguousarray(x[i, 0], dtype=np.float32).reshape(H // 2, 2 * W)
        in_maps.append({"x": img, "cmm": cmm})
    res = run_bass_kernel_spmd(nc, in_maps, core_ids=list(range(N_CORES)))
    outs = [res.results[i]["out"].reshape(3, H, W)[None] for i in range(N_CORES)]
    return np.concatenate(outs, axis=0)



# revision 4
# speedup vs baseline: 2.2560x; 2.2560x over previous
"""Bilinear RGGB demosaic (Bayer -> RGB) on 8 Trainium2 NeuronCores.

Layout: batch image n -> core n. Per core, the [2048, 2048] mosaic is
processed in 9 bands of 128 row-pairs; SBUF partition p of a band holds
the row pair (2p, 2p+1) concatenated in the free dim, so every DRAM
transfer is 16 KiB-contiguous per partition. Column-direction stencil
taps are free-dim shifted APs; the two row-direction taps are produced
on the tensor engine as 0.25*(row_{p-1}+row_p) / 0.25*(row_p+row_{p+1})
via banded 128x128 fp32 matmuls. Outputs are written bf16 (tolerance is
2e-2; bf16 rounds at ~4e-3) halving store traffic; the host widens back
to f32. Input DMAs ride the SP HWDGE ring (nc.sync), output DMAs the
ACT ring (nc.scalar) so loads never queue behind stores that are still
waiting on compute semaphores. Elementwise work is split DVE/ACT only
(GpSimd tensor ops run ~8 G elem/s and serialize the band pipeline).
"""

import sys

sys.path.insert(0, "/opt/trn_rl_repo")

import numpy as np

import concourse.bass as bass
import concourse.tile as tile
from concourse import mybir
from concourse.alu_op_type import AluOpType
from concourse.bass_utils import run_bass_kernel_spmd

F32 = mybir.dt.float32
BF16 = mybir.dt.bfloat16
MM_DT = mybir.dt.float32
N_CORES = 8
H = 2048
W = 2048


def split_sync_waits(nc, max_waits=1):
    """This walrus build rejects instructions carrying more than
    `max_waits` sync-wait commands. Hoist excess waits onto same-engine
    NoOps inserted immediately before the over-subscribed instruction
    (waiting earlier on the same queue is semantically conservative)."""
    for fn in nc.m.functions:
        for bb in fn.blocks:
            insts = bb.instructions
            i = 0
            while i < len(insts):
                inst = insts[i]
                si = inst.sync_info
                waits = list(si.on_wait) if si and si.on_wait else []
                if len(waits) > max_waits:
                    si.on_wait = waits[:max_waits]
                    excess = waits[max_waits:]
                    for j in range(0, len(excess), max_waits):
                        nop = mybir.InstNoOp(
                            name=nc.get_next_instruction_name(), ins=[], outs=[]
                        )
                        nop.engine = inst.engine
                        nop.sync_info = mybir.SyncInfo(
                            on_wait=excess[j : j + max_waits], on_update=[]
                        )
                        nc.register_instruction(nop)
                        insts.insert(i, nop)
                        i += 1
                i += 1


def const_arrays():
    m1 = 0.25 * (np.eye(128, dtype=np.float32) + np.eye(128, k=1, dtype=np.float32))
    m2 = 0.25 * (np.eye(128, dtype=np.float32) + np.eye(128, k=-1, dtype=np.float32))
    cmm = np.concatenate([m1, m2], axis=1)  # [128, 256]
    return cmm


def band_plan(npairs):
    """Bands of 128 row-pairs advancing ~126 pairs: each band stores only
    the pair range whose vertical neighbors are in-tile, so no halo or
    seam-fix work is needed. Returns [(start_pair, store_lo, store_hi)]."""
    plan = []
    covered = 0
    while covered < npairs:
        q = 0 if covered == 0 else min(covered - 1, npairs - 128)
        lo = covered - q
        hi = 128 if q + 128 >= npairs else 127
        plan.append((q, lo, hi))
        covered = q + hi
    return plan


def build_program(npairs=H // 2, w=W):
    nc = bass.Bass("TRN2", target_bir_lowering=False, debug=False)
    x = nc.dram_tensor("x", [npairs, 2 * w], F32, kind="ExternalInput").ap()
    cmm = nc.dram_tensor("cmm", [128, 256], F32, kind="ExternalInput").ap()
    out = nc.dram_tensor("out", [3, npairs, 2 * w], BF16, kind="ExternalOutput").ap()

    cw = 512  # matmul free-dim chunk (one PSUM bank of fp32)
    hw = w // 2
    plan = band_plan(npairs)

    with tile.TileContext(nc) as tc:
        with (
            tc.tile_pool(name="consts", bufs=1) as cpool,
            tc.tile_pool(name="inp", bufs=3) as ipool,
            tc.tile_pool(name="psum", bufs=1, space="PSUM") as ppool,
            tc.tile_pool(name="mids", bufs=2) as mpool,
            tc.tile_pool(name="tmps", bufs=2) as tpool,
            tc.tile_pool(name="outs", bufs=3) as opool,
        ):
            cM = cpool.tile([128, 256], F32)
            nc.sync.dma_start(cM[:], cmm[:])

            for q, lo, hi in plan:
                IN = ipool.tile([128, 2 * w], F32, tag="in")
                nc.sync.dma_start(IN[:], x[q : q + 128, :])

                E = IN[:, 0:w]
                O = IN[:, w : 2 * w]

                # Su4[p] = 0.25*(O[p-1] + O[p]) ; Sd4[p] = 0.25*(E[p] + E[p+1])
                # One ldweights per stationary: all Su chunks, then all Sd.
                Su4 = mpool.tile([128, w], F32, tag="su_sb")
                Sd4 = mpool.tile([128, w], F32, tag="sd_sb")
                mm = lambda ap: ap.bitcast(MM_DT)
                Sup0 = ppool.tile([128, 2 * cw], F32, tag="su0")
                Sup1 = ppool.tile([128, 2 * cw], F32, tag="su1")
                Sdp0 = ppool.tile([128, 2 * cw], F32, tag="sd0")
                Sdp1 = ppool.tile([128, 2 * cw], F32, tag="sd1")
                Sup = [Sup0, Sup1]
                Sdp = [Sdp0, Sdp1]
                for h in range(2):
                    for c in range(2):
                        s = slice(h * 2 * cw + c * cw, h * 2 * cw + (c + 1) * cw)
                        nc.tensor.matmul(
                            Sup[h][:, c * cw : (c + 1) * cw],
                            mm(cM[:, 0:128]), mm(O[:, s]),
                            start=True, stop=True,
                        )
                for h in range(2):
                    for c in range(2):
                        s = slice(h * 2 * cw + c * cw, h * 2 * cw + (c + 1) * cw)
                        nc.tensor.matmul(
                            Sdp[h][:, c * cw : (c + 1) * cw],
                            mm(cM[:, 128:256]), mm(E[:, s]),
                            start=True, stop=True,
                        )
                # PSUM -> SBUF: Su halves on ACT, Sd halves on DVE.
                for h in range(2):
                    nc.scalar.copy(Su4[:, h * 2 * cw : (h + 1) * 2 * cw], Sup[h][:])
                    nc.vector.tensor_copy(Sd4[:, h * 2 * cw : (h + 1) * 2 * cw], Sdp[h][:])

                Rt = opool.tile([128, 2 * w], BF16, tag="r")
                Gt = opool.tile([128, 2 * w], BF16, tag="g")
                Bt = opool.tile([128, 2 * w], BF16, tag="b")

                # ---- R channel ----
                # even rows, even cols: passthrough E
                nc.scalar.copy(Rt[:, 0:w:2], E[:, 0:w:2])
                # even rows, odd cols: 0.5*(E[x-1] + E[x+1])
                te = tpool.tile([128, hw], F32, tag="te")
                nc.vector.tensor_add(te[:, 0 : hw - 1], E[:, 0 : w - 2 : 2], E[:, 2:w:2])
                nc.scalar.mul(Rt[:, 1 : w - 2 : 2], te[:, 0 : hw - 1], 0.5)
                nc.vector.tensor_scalar_mul(
                    Rt[:, w - 1 : w], E[:, w - 2 : w - 1], 0.5
                )
                # odd rows, even cols: 2*Sd4
                nc.scalar.mul(Rt[:, w : 2 * w : 2], Sd4[:, 0:w:2], 2.0)
                # odd rows, odd cols: Sd4[x-1] + Sd4[x+1]
                nc.vector.tensor_add(
                    Rt[:, w + 1 : 2 * w - 2 : 2], Sd4[:, 0 : w - 2 : 2], Sd4[:, 2:w:2]
                )
                nc.vector.tensor_copy(
                    Rt[:, 2 * w - 1 : 2 * w], Sd4[:, w - 2 : w - 1]
                )
                nc.scalar.dma_start(out[0, q + lo : q + hi, :], Rt[lo:hi, :])

                # ---- G channel ----
                # even rows, even cols: 0.25*(E[x-1]+E[x+1]) + Su4[x]
                tg = tpool.tile([128, hw], F32, tag="tg")
                nc.vector.tensor_add(tg[:, 0 : hw - 1], E[:, 1 : w - 2 : 2], E[:, 3:w:2])
                nc.vector.scalar_tensor_tensor(
                    Gt[:, 2 : w - 1 : 2], tg[:, 0 : hw - 1], 0.25,
                    Su4[:, 2 : w - 1 : 2], AluOpType.mult, AluOpType.add,
                )
                nc.vector.scalar_tensor_tensor(
                    Gt[:, 0:1], E[:, 1:2], 0.25, Su4[:, 0:1],
                    AluOpType.mult, AluOpType.add,
                )
                # even rows, odd cols: passthrough E
                nc.scalar.copy(Gt[:, 1:w:2], E[:, 1:w:2])
                # odd rows, even cols: passthrough O
                nc.scalar.copy(Gt[:, w : 2 * w : 2], O[:, 0:w:2])
                # odd rows, odd cols: 0.25*(O[x-1]+O[x+1]) + Sd4[x]
                tg2 = tpool.tile([128, hw], F32, tag="tg2")
                nc.vector.tensor_add(
                    tg2[:, 0 : hw - 1], O[:, 0 : w - 2 : 2], O[:, 2:w:2]
                )
                nc.vector.scalar_tensor_tensor(
                    Gt[:, w + 1 : 2 * w - 2 : 2], tg2[:, 0 : hw - 1], 0.25,
                    Sd4[:, 1 : w - 2 : 2], AluOpType.mult, AluOpType.add,
                )
                nc.vector.scalar_tensor_tensor(
                    Gt[:, 2 * w - 1 : 2 * w], O[:, w - 2 : w - 1], 0.25,
                    Sd4[:, w - 1 : w], AluOpType.mult, AluOpType.add,
                )
                nc.scalar.dma_start(out[1, q + lo : q + hi, :], Gt[lo:hi, :])

                # ---- B channel ----
                # even rows, even cols: Su4[x-1] + Su4[x+1]
                nc.vector.tensor_add(
                    Bt[:, 2 : w - 1 : 2], Su4[:, 1 : w - 2 : 2], Su4[:, 3:w:2]
                )
                nc.vector.tensor_copy(Bt[:, 0:1], Su4[:, 1:2])
                # even rows, odd cols: 2*Su4
                nc.scalar.mul(Bt[:, 1:w:2], Su4[:, 1:w:2], 2.0)
                # odd rows, even cols: 0.5*(O[x-1]+O[x+1])
                tb = tpool.tile([128, hw], F32, tag="tb")
                nc.vector.tensor_add(tb[:, 0 : hw - 1], O[:, 1 : w - 2 : 2], O[:, 3:w:2])
                nc.scalar.mul(Bt[:, w + 2 : 2 * w - 1 : 2], tb[:, 0 : hw - 1], 0.5)
                nc.vector.tensor_scalar_mul(Bt[:, w : w + 1], O[:, 1:2], 0.5)
                # odd rows, odd cols: passthrough O
                nc.scalar.copy(Bt[:, w + 1 : 2 * w : 2], O[:, 1:w:2])
                nc.scalar.dma_start(out[2, q + lo : q + hi, :], Bt[lo:hi, :])

    split_sync_waits(nc)
    return nc


_CACHE = {}


def _get_program(npairs, w):
    key = (npairs, w)
    if key not in _CACHE:
        _CACHE[key] = build_program(npairs, w)
    return _CACHE[key]


def kernel(x: np.ndarray) -> np.ndarray:
    n, _, h, w = x.shape
    assert (n, h, w) == (N_CORES, H, W), x.shape
    nc = _get_program(H // 2, W)
    cmm = const_arrays()
    in_maps = []
    for i in range(N_CORES):
        img = np.ascontiguousarray(x[i, 0], dtype=np.float32).reshape(H // 2, 2 * W)
        in_maps.append({"x": img, "cmm": cmm})
    res = run_bass_kernel_spmd(nc, in_maps, core_ids=list(range(N_CORES)))
    outs = [
        np.asarray(res.results[i]["out"]).astype(np.float32).reshape(3, H, W)[None]
        for i in range(N_CORES)
    ]
    return np.concatenate(outs, axis=0)


# revision 9
# speedup vs baseline: 3.3391x; 1.4801x over previous
"""Bilinear RGGB demosaic (Bayer -> RGB) on 8 Trainium2 NeuronCores.

Layout: batch image n -> core n. Per core, the [2048, 2048] mosaic is
processed in 9 bands of 128 row-pairs; SBUF partition p of a band holds
the row pair (2p, 2p+1) concatenated in the free dim, so every DRAM
transfer is 16 KiB-contiguous per partition. Column-direction stencil
taps are free-dim shifted APs; the two row-direction taps are produced
on the tensor engine as 0.25*(row_{p-1}+row_p) / 0.25*(row_p+row_{p+1})
via banded 128x128 fp32 matmuls. Outputs are written bf16 (tolerance is
2e-2; bf16 rounds at ~4e-3) halving store traffic; the host widens back
to f32. Input DMAs ride the SP HWDGE ring (nc.sync), output DMAs the
ACT ring (nc.scalar) so loads never queue behind stores that are still
waiting on compute semaphores. Elementwise work is split DVE/ACT only
(GpSimd tensor ops run ~8 G elem/s and serialize the band pipeline).
"""

import sys

sys.path.insert(0, "/opt/trn_rl_repo")

import numpy as np

import concourse.bass as bass
import concourse.tile as tile
from concourse import mybir
from concourse.alu_op_type import AluOpType
from concourse.bass_utils import run_bass_kernel_spmd

F32 = mybir.dt.float32
BF16 = mybir.dt.bfloat16
MM_DT = mybir.dt.float32
N_CORES = 8
H = 2048
W = 2048


def split_sync_waits(nc, max_waits=1):
    """This walrus build rejects instructions carrying more than
    `max_waits` sync-wait commands. Hoist excess waits onto same-engine
    NoOps inserted immediately before the over-subscribed instruction
    (waiting earlier on the same queue is semantically conservative)."""
    for fn in nc.m.functions:
        for bb in fn.blocks:
            insts = bb.instructions
            i = 0
            while i < len(insts):
                inst = insts[i]
                si = inst.sync_info
                waits = list(si.on_wait) if si and si.on_wait else []
                if len(waits) > max_waits:
                    si.on_wait = waits[:max_waits]
                    excess = waits[max_waits:]
                    for j in range(0, len(excess), max_waits):
                        nop = mybir.InstNoOp(
                            name=nc.get_next_instruction_name(), ins=[], outs=[]
                        )
                        nop.engine = inst.engine
                        nop.sync_info = mybir.SyncInfo(
                            on_wait=excess[j : j + max_waits], on_update=[]
                        )
                        nc.register_instruction(nop)
                        insts.insert(i, nop)
                        i += 1
                i += 1


def const_arrays():
    m1 = 0.25 * (np.eye(128, dtype=np.float32) + np.eye(128, k=1, dtype=np.float32))
    m2 = 0.25 * (np.eye(128, dtype=np.float32) + np.eye(128, k=-1, dtype=np.float32))
    cmm = np.concatenate([m1, m2], axis=1)  # [128, 256]
    return cmm


def band_plan(npairs):
    """Bands of 128 row-pairs advancing ~126 pairs: each band stores only
    the pair range whose vertical neighbors are in-tile, so no halo or
    seam-fix work is needed. Returns [(start_pair, store_lo, store_hi)]."""
    plan = []
    covered = 0
    while covered < npairs:
        q = 0 if covered == 0 else min(covered - 1, npairs - 128)
        lo = covered - q
        hi = 128 if q + 128 >= npairs else 127
        plan.append((q, lo, hi))
        covered = q + hi
    return plan


def build_program(npairs=H // 2, w=W):
    nc = bass.Bass("TRN2", target_bir_lowering=False, debug=False)
    x = nc.dram_tensor("x", [npairs, 2 * w], F32, kind="ExternalInput").ap()
    cmm = nc.dram_tensor("cmm", [128, 256], F32, kind="ExternalInput").ap()
    out = nc.dram_tensor("out", [3, npairs, 2 * w], BF16, kind="ExternalOutput").ap()

    cw = 512  # matmul free-dim chunk (one PSUM bank of fp32)
    hw = w // 2
    plan = band_plan(npairs)

    with tile.TileContext(nc) as tc:
        with (
            tc.tile_pool(name="consts", bufs=1) as cpool,
            tc.tile_pool(name="inp", bufs=3) as ipool,
            tc.tile_pool(name="psum", bufs=1, space="PSUM") as ppool,
            tc.tile_pool(name="mids", bufs=2) as mpool,
            tc.tile_pool(name="tmps", bufs=2) as tpool,
            tc.tile_pool(name="outs", bufs=3) as opool,
        ):
            cM = cpool.tile([128, 256], F32)
            nc.sync.dma_start(cM[:], cmm[:])

            for q, lo, hi in plan:
                IN = ipool.tile([128, 2 * w], F32, tag="in")
                nc.sync.dma_start(IN[:], x[q : q + 128, :])

                E = IN[:, 0:w]
                O = IN[:, w : 2 * w]

                # Su4[p] = 0.25*(O[p-1] + O[p]) ; Sd4[p] = 0.25*(E[p] + E[p+1])
                # One ldweights per stationary: all Su chunks, then all Sd.
                Su4 = mpool.tile([128, w], F32, tag="su_sb")
                Sd4 = mpool.tile([128, w], F32, tag="sd_sb")
                mm = lambda ap: ap.bitcast(MM_DT)
                Sup0 = ppool.tile([128, 2 * cw], F32, tag="su0")
                Sup1 = ppool.tile([128, 2 * cw], F32, tag="su1")
                Sdp0 = ppool.tile([128, 2 * cw], F32, tag="sd0")
                Sdp1 = ppool.tile([128, 2 * cw], F32, tag="sd1")
                Sup = [Sup0, Sup1]
                Sdp = [Sdp0, Sdp1]
                for h in range(2):
                    for c in range(2):
                        s = slice(h * 2 * cw + c * cw, h * 2 * cw + (c + 1) * cw)
                        nc.tensor.matmul(
                            Sup[h][:, c * cw : (c + 1) * cw],
                            mm(cM[:, 0:128]), mm(O[:, s]),
                            start=True, stop=True,
                        )
                for h in range(2):
                    for c in range(2):
                        s = slice(h * 2 * cw + c * cw, h * 2 * cw + (c + 1) * cw)
                        nc.tensor.matmul(
                            Sdp[h][:, c * cw : (c + 1) * cw],
                            mm(cM[:, 128:256]), mm(E[:, s]),
                            start=True, stop=True,
                        )
                # PSUM -> SBUF: Su halves on ACT, Sd halves on DVE.
                for h in range(2):
                    nc.scalar.copy(Su4[:, h * 2 * cw : (h + 1) * 2 * cw], Sup[h][:])
                    nc.vector.tensor_copy(Sd4[:, h * 2 * cw : (h + 1) * 2 * cw], Sdp[h][:])

                Rt = opool.tile([128, 2 * w], BF16, tag="r")
                Gt = opool.tile([128, 2 * w], BF16, tag="g")
                Bt = opool.tile([128, 2 * w], BF16, tag="b")

                # ---- R channel ----
                # even rows, even cols: passthrough E
                nc.scalar.copy(Rt[:, 0:w:2], E[:, 0:w:2])
                # even rows, odd cols: 0.5*(E[x-1] + E[x+1])
                te = tpool.tile([128, hw], F32, tag="te")
                nc.vector.tensor_add(te[:, 0 : hw - 1], E[:, 0 : w - 2 : 2], E[:, 2:w:2])
                nc.scalar.mul(Rt[:, 1 : w - 2 : 2], te[:, 0 : hw - 1], 0.5)
                nc.vector.tensor_scalar_mul(
                    Rt[:, w - 1 : w], E[:, w - 2 : w - 1], 0.5
                )
                # odd rows, even cols: 2*Sd4
                nc.scalar.mul(Rt[:, w : 2 * w : 2], Sd4[:, 0:w:2], 2.0)
                # odd rows, odd cols: Sd4[x-1] + Sd4[x+1]
                nc.vector.tensor_add(
                    Rt[:, w + 1 : 2 * w - 2 : 2], Sd4[:, 0 : w - 2 : 2], Sd4[:, 2:w:2]
                )
                nc.vector.tensor_copy(
                    Rt[:, 2 * w - 1 : 2 * w], Sd4[:, w - 2 : w - 1]
                )
                nc.gpsimd.dma_start(out[0, q + lo : q + hi, :], Rt[lo:hi, :])

                # ---- G channel ----
                # even rows, even cols: 0.25*(E[x-1]+E[x+1]) + Su4[x]
                tg = tpool.tile([128, hw], F32, tag="tg")
                nc.vector.tensor_add(tg[:, 0 : hw - 1], E[:, 1 : w - 2 : 2], E[:, 3:w:2])
                nc.vector.scalar_tensor_tensor(
                    Gt[:, 2 : w - 1 : 2], tg[:, 0 : hw - 1], 0.25,
                    Su4[:, 2 : w - 1 : 2], AluOpType.mult, AluOpType.add,
                )
                nc.vector.scalar_tensor_tensor(
                    Gt[:, 0:1], E[:, 1:2], 0.25, Su4[:, 0:1],
                    AluOpType.mult, AluOpType.add,
                )
                # even rows, odd cols: passthrough E
                nc.scalar.copy(Gt[:, 1:w:2], E[:, 1:w:2])
                # odd rows, even cols: passthrough O (DVE 2x_2P single-src)
                nc.vector.tensor_copy(Gt[:, w : 2 * w : 2], O[:, 0:w:2])
                # odd rows, odd cols: 0.25*(O[x-1]+O[x+1]) + Sd4[x]
                tg2 = tpool.tile([128, hw], F32, tag="tg2")
                nc.vector.tensor_add(
                    tg2[:, 0 : hw - 1], O[:, 0 : w - 2 : 2], O[:, 2:w:2]
                )
                nc.vector.scalar_tensor_tensor(
                    Gt[:, w + 1 : 2 * w - 2 : 2], tg2[:, 0 : hw - 1], 0.25,
                    Sd4[:, 1 : w - 2 : 2], AluOpType.mult, AluOpType.add,
                )
                nc.vector.scalar_tensor_tensor(
                    Gt[:, 2 * w - 1 : 2 * w], O[:, w - 2 : w - 1], 0.25,
                    Sd4[:, w - 1 : w], AluOpType.mult, AluOpType.add,
                )
                nc.gpsimd.dma_start(out[1, q + lo : q + hi, :], Gt[lo:hi, :])

                # ---- B channel ----
                # even rows, even cols: Su4[x-1] + Su4[x+1]
                nc.vector.tensor_add(
                    Bt[:, 2 : w - 1 : 2], Su4[:, 1 : w - 2 : 2], Su4[:, 3:w:2]
                )
                nc.vector.tensor_copy(Bt[:, 0:1], Su4[:, 1:2])
                # even rows, odd cols: 2*Su4 (DVE 2x_2P single-src)
                nc.vector.tensor_scalar_mul(Bt[:, 1:w:2], Su4[:, 1:w:2], 2.0)
                # odd rows, even cols: 0.5*(O[x-1]+O[x+1])
                tb = tpool.tile([128, hw], F32, tag="tb")
                nc.vector.tensor_add(tb[:, 0 : hw - 1], O[:, 1 : w - 2 : 2], O[:, 3:w:2])
                nc.scalar.mul(Bt[:, w + 2 : 2 * w - 1 : 2], tb[:, 0 : hw - 1], 0.5)
                nc.vector.tensor_scalar_mul(Bt[:, w : w + 1], O[:, 1:2], 0.5)
                # odd rows, odd cols: passthrough O (DVE 2x_2P single-src)
                nc.vector.tensor_copy(Bt[:, w + 1 : 2 * w : 2], O[:, 1:w:2])
                nc.gpsimd.dma_start(out[2, q + lo : q + hi, :], Bt[lo:hi, :])

    split_sync_waits(nc)
    return nc


_CACHE = {}


def _get_program(npairs, w):
    key = (npairs, w)
    if key not in _CACHE:
        _CACHE[key] = build_program(npairs, w)
    return _CACHE[key]


def kernel(x: np.ndarray) -> np.ndarray:
    n, _, h, w = x.shape
    assert (n, h, w) == (N_CORES, H, W), x.shape
    nc = _get_program(H // 2, W)
    cmm = const_arrays()
    in_maps = []
    for i in range(N_CORES):
        img = np.ascontiguousarray(x[i, 0], dtype=np.float32).reshape(H // 2, 2 * W)
        in_maps.append({"x": img, "cmm": cmm})
    res = run_bass_kernel_spmd(nc, in_maps, core_ids=list(range(N_CORES)))
    outs = [
        np.asarray(res.results[i]["out"]).astype(np.float32).reshape(3, H, W)[None]
        for i in range(N_CORES)
    ]
    return np.concatenate(outs, axis=0)


# revision 14
# speedup vs baseline: 4.2905x; 1.2849x over previous
"""Bilinear RGGB demosaic (Bayer -> RGB) on 8 Trainium2 NeuronCores.

Layout: batch image n -> core n. Per core, the [2048, 2048] mosaic is
processed in 8 bands of exactly 128 row-pairs; SBUF partition p of a
band holds the row pair (2p, 2p+1) concatenated in the free dim, so
every DRAM transfer is 16 KiB-contiguous per partition.

Vertical stencil taps are banded 128x128 matmuls (fp32r) on the tensor
engine; band-seam rows are fixed up with K=1 matmuls against the
neighbor band's input tile (accumulated into the same PSUM bank, so no
halo DMA and no overlap bands). The G channel is computed ENTIRELY on
the tensor engine: its horizontal taps are expressed as additional
accumulating matmuls whose moving operands are column-shifted APs of
the same input tile (PSUM accumulation = free adds). Su4/Sd4 are only
produced at the column parity their R/B consumers need, which also
makes every remaining DVE add contiguous.

Outputs are written bf16 (tolerance 2e-2; bf16 rounds at ~2e-3),
halving store traffic; the host widens to f32. Input DMAs ride the SP
HWDGE ring (nc.sync); output DMAs are issued by the otherwise-idle
GpSimd SWDGE so neither compute engine's instruction stream ever
blocks on a store. Elementwise work is balanced across DVE and ACT.
"""

import sys

sys.path.insert(0, "/opt/trn_rl_repo")

import numpy as np

import concourse.bass as bass
import concourse.tile as tile
from concourse import mybir
from concourse.alu_op_type import AluOpType
from concourse.bass_utils import run_bass_kernel_spmd

F32 = mybir.dt.float32
BF16 = mybir.dt.bfloat16
N_CORES = 8
H = 2048
W = 2048
NBANDS = H // 256  # 8 bands of 128 row-pairs


def split_sync_waits(nc, max_waits=1):
    """This walrus build rejects instructions carrying more than
    `max_waits` sync-wait commands. Hoist excess waits onto same-engine
    NoOps inserted immediately before the over-subscribed instruction
    (waiting earlier on the same queue is semantically conservative)."""
    for fn in nc.m.functions:
        for bb in fn.blocks:
            insts = bb.instructions
            i = 0
            while i < len(insts):
                inst = insts[i]
                si = inst.sync_info
                waits = list(si.on_wait) if si and si.on_wait else []
                if len(waits) > max_waits:
                    si.on_wait = waits[:max_waits]
                    excess = waits[max_waits:]
                    for j in range(0, len(excess), max_waits):
                        nop = mybir.InstNoOp(
                            name=nc.get_next_instruction_name(), ins=[], outs=[]
                        )
                        nop.engine = inst.engine
                        nop.sync_info = mybir.SyncInfo(
                            on_wait=excess[j : j + max_waits], on_update=[]
                        )
                        nc.register_instruction(nop)
                        insts.insert(i, nop)
                        i += 1
                i += 1


def const_arrays():
    # cmm[:, 0:128]   m1 : Su[p] = 0.25*(O[p-1] + O[p])
    # cmm[:, 128:256] m2 : Sd[p] = 0.25*(E[p] + E[p+1])
    # cmm[:, 256:384] qI : 0.25 * I (horizontal quarter taps)
    import ml_dtypes

    m1 = 0.25 * (np.eye(128, dtype=np.float32) + np.eye(128, k=1, dtype=np.float32))
    m2 = 0.25 * (np.eye(128, dtype=np.float32) + np.eye(128, k=-1, dtype=np.float32))
    qI = 0.25 * np.eye(128, dtype=np.float32)
    cmm = np.concatenate([m1, m2, qI], axis=1).astype(ml_dtypes.bfloat16)  # [128, 384]
    # cfx[0, 0:128]   fu : row vector, 0.25 into partition 0   (+= 0.25*O_prev)
    # cfx[0, 128:256] fd : row vector, 0.25 into partition 127 (+= 0.25*E_next)
    cfx = np.zeros((1, 256), dtype=np.float32)
    cfx[0, 0] = 0.25
    cfx[0, 128 + 127] = 0.25
    return cmm, cfx.astype(ml_dtypes.bfloat16)


def build_program(npairs=H // 2, w=W):
    nc = bass.Bass("TRN2", target_bir_lowering=False, debug=False)
    x = nc.dram_tensor("x", [npairs, 2 * w], F32, kind="ExternalInput").ap()
    cmm = nc.dram_tensor("cmm", [128, 384], BF16, kind="ExternalInput").ap()
    cfx = nc.dram_tensor("cfx", [1, 256], BF16, kind="ExternalInput").ap()
    out = nc.dram_tensor("out", [3, npairs, 2 * w], BF16, kind="ExternalOutput").ap()

    hw = w // 2  # 1024: outputs of one parity
    qw = w // 4  # 512: one PSUM bank / one half-band of one parity
    nb = npairs // 128
    assert nb * 128 == npairs

    with tile.TileContext(nc) as tc:
        with (
            tc.tile_pool(name="consts", bufs=1) as cpool,
            tc.tile_pool(name="inp", bufs=3) as ipool,
            tc.tile_pool(name="psum", bufs=1, space="PSUM") as ppool,
            tc.tile_pool(name="mids", bufs=2) as mpool,
            tc.tile_pool(name="tmps", bufs=2) as tpool,
            tc.tile_pool(name="outs", bufs=3) as opool,
        ):
            cM = cpool.tile([128, 384], BF16)
            nc.sync.dma_start(cM[:], cmm[:])
            cF = cpool.tile([1, 256], BF16)
            nc.sync.dma_start(cF[:], cfx[:])

            mm = lambda ap: ap
            m1 = mm(cM[:, 0:128])
            m2 = mm(cM[:, 128:256])
            qI = mm(cM[:, 256:384])
            fu = mm(cF[:, 0:128])
            fd = mm(cF[:, 128:256])

            INs = {}

            def get_in(k):
                if k not in INs:
                    t = ipool.tile([128, 2 * w], BF16, tag="in")
                    nc.gpsimd.dma_start(t[:], x[128 * k : 128 * k + 128, :])
                    INs[k] = t
                return INs[k]

            for k in range(nb):
                IN = get_in(k)
                INn = get_in(k + 1) if k + 1 < nb else None

                E = IN[:, 0:w]
                O = IN[:, w : 2 * w]
                # halo rows, single partition each. The next band's first E
                # row is partition 0 of its input tile (a legal matmul base
                # partition); the previous band's last O row would sit at
                # partition 127, so re-fetch those 8 KiB into partition 0.
                Op = None
                if k > 0:
                    OpH = tpool.tile([1, w], BF16, tag="oph")
                    nc.gpsimd.dma_start(OpH[:], x[128 * k - 1 : 128 * k, w : 2 * w])
                    Op = OpH[:]
                En = INn[0:1, 0:w] if INn is not None else None

                # PSUM, per half h (columns [1024h, 1024h+1024)):
                #   SuP[h] [128,512]: Su4 at odd cols of the half
                #   SdP[h] [128,512]: Su4 at even cols
                #   GeP[h] [128,512]: G at (even row, even col)
                #   GoP[h] [128,512]: G at (odd row, odd col)
                SuP0 = ppool.tile([128, qw], F32, tag="su0")
                SuP1 = ppool.tile([128, qw], F32, tag="su1")
                SdP0 = ppool.tile([128, qw], F32, tag="sd0")
                SdP1 = ppool.tile([128, qw], F32, tag="sd1")
                GeP0 = ppool.tile([128, qw], F32, tag="ge0")
                GeP1 = ppool.tile([128, qw], F32, tag="ge1")
                GoP0 = ppool.tile([128, qw], F32, tag="go0")
                GoP1 = ppool.tile([128, qw], F32, tag="go1")
                SuP = [SuP0, SuP1]
                SdP = [SdP0, SdP1]
                GeP = [GeP0, GeP1]
                GoP = [GoP0, GoP1]

                MM = nc.tensor.matmul
                for h in range(2):
                    c = hw * h  # column base of this half
                    # --- stationary m1 ---
                    # Su4 @ odd cols; G(e,e) vertical part = Su4 @ even cols
                    MM(SuP[h][:], m1, mm(O[:, c + 1 : c + hw : 2]),
                       start=True, stop=(Op is None))
                    MM(GeP[h][:], m1, mm(O[:, c : c + hw : 2]),
                       start=True, stop=False)
                    # --- stationary m2 ---
                    MM(SdP[h][:], m2, mm(E[:, c : c + hw : 2]),
                       start=True, stop=(INn is None))
                    MM(GoP[h][:], m2, mm(E[:, c + 1 : c + hw : 2]),
                       start=True, stop=False)
                    # --- stationary qI: horizontal quarter taps ---
                    # G(e,e)[x] += 0.25*(E[x-1] + E[x+1]), x even
                    if c == 0:
                        MM(GeP[h][:, 1:qw], qI, mm(E[:, 1 : hw - 1 : 2]),
                           start=False, stop=False, skip_group_check=True)
                    else:
                        MM(GeP[h][:], qI, mm(E[:, c - 1 : c + hw - 1 : 2]),
                           start=False, stop=False, skip_group_check=True)
                    MM(GeP[h][:], qI, mm(E[:, c + 1 : min(c + hw + 1, w) : 2]),
                       start=False, stop=(Op is None), skip_group_check=True)
                    # G(o,o)[x] += 0.25*(O[x-1] + O[x+1]), x odd
                    MM(GoP[h][:], qI, mm(O[:, c : c + hw : 2]),
                       start=False, stop=False, skip_group_check=True)
                    if c + hw == w:
                        MM(GoP[h][:, 0 : qw - 1], qI, mm(O[:, c + 2 : c + hw : 2]),
                           start=False, stop=(INn is None), skip_group_check=True)
                    else:
                        MM(GoP[h][:], qI, mm(O[:, c + 2 : c + hw + 2 : 2]),
                           start=False, stop=(INn is None), skip_group_check=True)
                    # --- seam fixups (K=1 matmuls against neighbor tiles) ---
                    if Op is not None:
                        MM(SuP[h][:], fu, mm(Op[:, c + 1 : c + hw : 2]),
                           start=False, stop=True, skip_group_check=True)
                        MM(GeP[h][:], fu, mm(Op[:, c : c + hw : 2]),
                           start=False, stop=True, skip_group_check=True)
                    if INn is not None:
                        MM(SdP[h][:], fd, mm(En[:, c : c + hw : 2]),
                           start=False, stop=True, skip_group_check=True)
                        MM(GoP[h][:], fd, mm(En[:, c + 1 : c + hw : 2]),
                           start=False, stop=True, skip_group_check=True)

                # Su4o[j] = Su4[2j+1], Sd4e[j] = Sd4[2j]   (contiguous, f32)
                Su4o = mpool.tile([128, hw], F32, tag="su4o")
                Sd4e = mpool.tile([128, hw], F32, tag="sd4e")
                for h in range(2):
                    nc.scalar.copy(Su4o[:, h * qw : (h + 1) * qw], SuP[h][:])
                    nc.scalar.copy(Sd4e[:, h * qw : (h + 1) * qw], SdP[h][:])

                Rt = opool.tile([128, 2 * w], BF16, tag="r")
                Gt = opool.tile([128, 2 * w], BF16, tag="g")
                Bt = opool.tile([128, 2 * w], BF16, tag="b")

                # ---- G channel ----
                # (e,e) and (o,o) straight from PSUM (strided bf16 writes)
                for h in range(2):
                    nc.scalar.copy(Gt[:, 2 * h * qw : 2 * (h + 1) * qw : 2], GeP[h][:])
                    nc.scalar.copy(
                        Gt[:, w + 2 * h * qw + 1 : w + 2 * (h + 1) * qw : 2], GoP[h][:]
                    )
                # (e,o): passthrough E odd cols; (o,e): passthrough O even cols
                nc.scalar.copy(Gt[:, 1:w:2], E[:, 1:w:2])
                nc.vector.tensor_copy(Gt[:, w : 2 * w : 2], O[:, 0:w:2])
                nc.sync.dma_start(out[1, 128 * k : 128 * k + 128, :], Gt[:])

                # ---- R channel ----
                # (e,e): passthrough E even cols
                nc.scalar.copy(Rt[:, 0:w:2], E[:, 0:w:2])
                # (e,o): 0.5*(E[x-1] + E[x+1]); te padded to hw, col w-1 fixed after
                te = tpool.tile([128, hw], F32, tag="te")
                nc.vector.tensor_add(te[:, 0 : hw - 1], E[:, 0 : w - 2 : 2], E[:, 2:w:2])
                nc.vector.tensor_copy(te[:, hw - 1 : hw], E[:, w - 2 : w - 1])
                nc.vector.tensor_scalar_mul(Rt[:, 1:w:2], te[:], 0.5)
                # (o,e): 2*Sd4 at even cols
                nc.scalar.mul(Rt[:, w : 2 * w : 2], Sd4e[:], 2.0)
                # (o,o): Sd4[x-1] + Sd4[x+1] = Sd4e[j] + Sd4e[j+1]; last col copy
                nc.vector.tensor_add(
                    Rt[:, w + 1 : 2 * w - 2 : 2], Sd4e[:, 0 : hw - 1], Sd4e[:, 1:hw]
                )
                nc.vector.tensor_copy(
                    Rt[:, 2 * w - 1 : 2 * w], Sd4e[:, hw - 1 : hw]
                )
                nc.sync.dma_start(out[0, 128 * k : 128 * k + 128, :], Rt[:])

                # ---- B channel ----
                # (e,e): Su4[x-1] + Su4[x+1] = Su4o[j-1] + Su4o[j]; col 0 copy
                nc.vector.tensor_add(
                    Bt[:, 2 : w - 1 : 2], Su4o[:, 0 : hw - 1], Su4o[:, 1:hw]
                )
                nc.vector.tensor_copy(Bt[:, 0:1], Su4o[:, 0:1])
                # (e,o): 2*Su4 at odd cols
                nc.vector.tensor_scalar_mul(Bt[:, 1:w:2], Su4o[:], 2.0)
                # (o,e): 0.5*(O[x-1] + O[x+1]); tb padded, col w... col 0 fixed after
                tb = tpool.tile([128, hw], F32, tag="tb")
                nc.vector.tensor_add(tb[:, 1:hw], O[:, 1 : w - 2 : 2], O[:, 3:w:2])
                nc.vector.tensor_copy(tb[:, 0:1], O[:, 1:2])
                nc.scalar.mul(Bt[:, w : 2 * w : 2], tb[:], 0.5)
                # (o,o): passthrough O odd cols
                nc.vector.tensor_copy(Bt[:, w + 1 : 2 * w : 2], O[:, 1:w:2])
                nc.sync.dma_start(out[2, 128 * k : 128 * k + 128, :], Bt[:])

                if k - 1 in INs:
                    del INs[k - 1]

    split_sync_waits(nc)
    return nc


_CACHE = {}


def _get_program(npairs, w):
    key = (npairs, w)
    if key not in _CACHE:
        _CACHE[key] = build_program(npairs, w)
    return _CACHE[key]


def kernel(x: np.ndarray) -> np.ndarray:
    n, _, h, w = x.shape
    assert (n, h, w) == (N_CORES, H, W), x.shape
    nc = _get_program(H // 2, W)
    cmm, cfx = const_arrays()
    in_maps = []
    for i in range(N_CORES):
        img = np.ascontiguousarray(x[i, 0], dtype=np.float32).reshape(H // 2, 2 * W)
        in_maps.append({"x": img, "cmm": cmm, "cfx": cfx})
    res = run_bass_kernel_spmd(nc, in_maps, core_ids=list(range(N_CORES)))
    outs = [
        np.asarray(res.results[i]["out"]).astype(np.float32).reshape(3, H, W)[None]
        for i in range(N_CORES)
    ]
    return np.concatenate(outs, axis=0)


# revision 15
# speedup vs baseline: 4.5177x; 1.0530x over previous
"""Bilinear RGGB demosaic (Bayer -> RGB) on 8 Trainium2 NeuronCores.

Layout: batch image n -> core n. Per core, the [2048, 2048] mosaic is
processed in 8 bands of exactly 128 row-pairs; SBUF partition p of a
band holds the row pair (2p, 2p+1) concatenated in the free dim, so
every DRAM transfer is 16 KiB-contiguous per partition.

Vertical stencil taps are banded 128x128 matmuls (fp32r) on the tensor
engine; band-seam rows are fixed up with K=1 matmuls against the
neighbor band's input tile (accumulated into the same PSUM bank, so no
halo DMA and no overlap bands). The G channel is computed ENTIRELY on
the tensor engine: its horizontal taps are expressed as additional
accumulating matmuls whose moving operands are column-shifted APs of
the same input tile (PSUM accumulation = free adds). Su4/Sd4 are only
produced at the column parity their R/B consumers need, which also
makes every remaining DVE add contiguous.

Outputs are written bf16 (tolerance 2e-2; bf16 rounds at ~2e-3),
halving store traffic; the host widens to f32. Input DMAs ride the SP
HWDGE ring (nc.sync); output DMAs are issued by the otherwise-idle
GpSimd SWDGE so neither compute engine's instruction stream ever
blocks on a store. Elementwise work is balanced across DVE and ACT.
"""

import sys

sys.path.insert(0, "/opt/trn_rl_repo")

import numpy as np

import concourse.bass as bass
import concourse.tile as tile
from concourse import mybir
from concourse.alu_op_type import AluOpType
from concourse.bass_utils import run_bass_kernel_spmd

F32 = mybir.dt.float32
BF16 = mybir.dt.bfloat16
N_CORES = 8
H = 2048
W = 2048
NBANDS = H // 256  # 8 bands of 128 row-pairs


def split_sync_waits(nc, max_waits=1):
    """This walrus build rejects instructions carrying more than
    `max_waits` sync-wait commands. Hoist excess waits onto same-engine
    NoOps inserted immediately before the over-subscribed instruction
    (waiting earlier on the same queue is semantically conservative)."""
    for fn in nc.m.functions:
        for bb in fn.blocks:
            insts = bb.instructions
            i = 0
            while i < len(insts):
                inst = insts[i]
                si = inst.sync_info
                waits = list(si.on_wait) if si and si.on_wait else []
                if len(waits) > max_waits:
                    si.on_wait = waits[:max_waits]
                    excess = waits[max_waits:]
                    for j in range(0, len(excess), max_waits):
                        nop = mybir.InstNoOp(
                            name=nc.get_next_instruction_name(), ins=[], outs=[]
                        )
                        nop.engine = inst.engine
                        nop.sync_info = mybir.SyncInfo(
                            on_wait=excess[j : j + max_waits], on_update=[]
                        )
                        nc.register_instruction(nop)
                        insts.insert(i, nop)
                        i += 1
                i += 1


def const_arrays():
    # cmm[:, 0:128]   m1 : Su[p] = 0.25*(O[p-1] + O[p])
    # cmm[:, 128:256] m2 : Sd[p] = 0.25*(E[p] + E[p+1])
    # cmm[:, 256:384] qI : 0.25 * I (horizontal quarter taps)
    import ml_dtypes

    m1 = 0.25 * (np.eye(128, dtype=np.float32) + np.eye(128, k=1, dtype=np.float32))
    m2 = 0.25 * (np.eye(128, dtype=np.float32) + np.eye(128, k=-1, dtype=np.float32))
    qI = 0.25 * np.eye(128, dtype=np.float32)
    cmm = np.concatenate([m1, m2, qI], axis=1).astype(ml_dtypes.bfloat16)  # [128, 384]
    # cfx[0, 0:128]   fu : row vector, 0.25 into partition 0   (+= 0.25*O_prev)
    # cfx[0, 128:256] fd : row vector, 0.25 into partition 127 (+= 0.25*E_next)
    cfx = np.zeros((1, 256), dtype=np.float32)
    cfx[0, 0] = 0.25
    cfx[0, 128 + 127] = 0.25
    return cmm, cfx.astype(ml_dtypes.bfloat16)


def build_program(npairs=H // 2, w=W):
    nc = bass.Bass("TRN2", target_bir_lowering=False, debug=False)
    x = nc.dram_tensor("x", [npairs, 2 * w], F32, kind="ExternalInput").ap()
    cmm = nc.dram_tensor("cmm", [128, 384], BF16, kind="ExternalInput").ap()
    cfx = nc.dram_tensor("cfx", [1, 256], BF16, kind="ExternalInput").ap()
    out = nc.dram_tensor("out", [3, npairs, 2 * w], BF16, kind="ExternalOutput").ap()

    hw = w // 2  # 1024: outputs of one parity
    qw = w // 4  # 512: one PSUM bank / one half-band of one parity
    nb = npairs // 128
    assert nb * 128 == npairs

    with tile.TileContext(nc) as tc:
        with (
            tc.tile_pool(name="consts", bufs=1) as cpool,
            tc.tile_pool(name="inp", bufs=3) as ipool,
            tc.tile_pool(name="psum", bufs=1, space="PSUM") as ppool,
            tc.tile_pool(name="mids", bufs=2) as mpool,
            tc.tile_pool(name="tmps", bufs=2) as tpool,
            tc.tile_pool(name="outs", bufs=3) as opool,
        ):
            cM = cpool.tile([128, 384], BF16)
            nc.sync.dma_start(cM[:], cmm[:])
            cF = cpool.tile([1, 256], BF16)
            nc.sync.dma_start(cF[:], cfx[:])

            mm = lambda ap: ap
            m1 = mm(cM[:, 0:128])
            m2 = mm(cM[:, 128:256])
            qI = mm(cM[:, 256:384])
            fu = mm(cF[:, 0:128])
            fd = mm(cF[:, 128:256])

            INs = {}

            def get_in(k):
                if k not in INs:
                    t = ipool.tile([128, 2 * w], BF16, tag="in")
                    nc.gpsimd.dma_start(t[:], x[128 * k : 128 * k + 128, :])
                    INs[k] = t
                return INs[k]

            for k in range(nb):
                IN = get_in(k)
                INn = get_in(k + 1) if k + 1 < nb else None

                E = IN[:, 0:w]
                O = IN[:, w : 2 * w]
                # halo rows, single partition each. The next band's first E
                # row is partition 0 of its input tile (a legal matmul base
                # partition); the previous band's last O row would sit at
                # partition 127, so re-fetch those 8 KiB into partition 0.
                Op = None
                if k > 0:
                    OpH = tpool.tile([1, w], BF16, tag="oph")
                    nc.gpsimd.dma_start(OpH[:], x[128 * k - 1 : 128 * k, w : 2 * w])
                    Op = OpH[:]
                En = INn[0:1, 0:w] if INn is not None else None

                # PSUM, per half h (columns [1024h, 1024h+1024)):
                #   SuP[h] [128,512]: Su4 at odd cols of the half
                #   SdP[h] [128,512]: Su4 at even cols
                #   GeP[h] [128,512]: G at (even row, even col)
                #   GoP[h] [128,512]: G at (odd row, odd col)
                SuP0 = ppool.tile([128, qw], F32, tag="su0")
                SuP1 = ppool.tile([128, qw], F32, tag="su1")
                SdP0 = ppool.tile([128, qw], F32, tag="sd0")
                SdP1 = ppool.tile([128, qw], F32, tag="sd1")
                GeP0 = ppool.tile([128, qw], F32, tag="ge0")
                GeP1 = ppool.tile([128, qw], F32, tag="ge1")
                GoP0 = ppool.tile([128, qw], F32, tag="go0")
                GoP1 = ppool.tile([128, qw], F32, tag="go1")
                SuP = [SuP0, SuP1]
                SdP = [SdP0, SdP1]
                GeP = [GeP0, GeP1]
                GoP = [GoP0, GoP1]

                MM = nc.tensor.matmul
                for h in range(2):
                    c = hw * h  # column base of this half
                    # --- stationary m1 ---
                    # Su4 @ odd cols; G(e,e) vertical part = Su4 @ even cols
                    MM(SuP[h][:], m1, mm(O[:, c + 1 : c + hw : 2]),
                       start=True, stop=(Op is None))
                    MM(GeP[h][:], m1, mm(O[:, c : c + hw : 2]),
                       start=True, stop=False)
                    # --- stationary m2 ---
                    MM(SdP[h][:], m2, mm(E[:, c : c + hw : 2]),
                       start=True, stop=(INn is None))
                    MM(GoP[h][:], m2, mm(E[:, c + 1 : c + hw : 2]),
                       start=True, stop=False)
                    # --- stationary qI: horizontal quarter taps ---
                    # G(e,e)[x] += 0.25*(E[x-1] + E[x+1]), x even
                    if c == 0:
                        MM(GeP[h][:, 1:qw], qI, mm(E[:, 1 : hw - 1 : 2]),
                           start=False, stop=False, skip_group_check=True)
                    else:
                        MM(GeP[h][:], qI, mm(E[:, c - 1 : c + hw - 1 : 2]),
                           start=False, stop=False, skip_group_check=True)
                    MM(GeP[h][:], qI, mm(E[:, c + 1 : min(c + hw + 1, w) : 2]),
                       start=False, stop=(Op is None), skip_group_check=True)
                    # G(o,o)[x] += 0.25*(O[x-1] + O[x+1]), x odd
                    MM(GoP[h][:], qI, mm(O[:, c : c + hw : 2]),
                       start=False, stop=False, skip_group_check=True)
                    if c + hw == w:
                        MM(GoP[h][:, 0 : qw - 1], qI, mm(O[:, c + 2 : c + hw : 2]),
                           start=False, stop=(INn is None), skip_group_check=True)
                    else:
                        MM(GoP[h][:], qI, mm(O[:, c + 2 : c + hw + 2 : 2]),
                           start=False, stop=(INn is None), skip_group_check=True)
                    # --- seam fixups (K=1 matmuls against neighbor tiles) ---
                    if Op is not None:
                        MM(SuP[h][:], fu, mm(Op[:, c + 1 : c + hw : 2]),
                           start=False, stop=True, skip_group_check=True)
                        MM(GeP[h][:], fu, mm(Op[:, c : c + hw : 2]),
                           start=False, stop=True, skip_group_check=True)
                    if INn is not None:
                        MM(SdP[h][:], fd, mm(En[:, c : c + hw : 2]),
                           start=False, stop=True, skip_group_check=True)
                        MM(GoP[h][:], fd, mm(En[:, c + 1 : c + hw : 2]),
                           start=False, stop=True, skip_group_check=True)

                # Su4o[j] = Su4[2j+1], Sd4e[j] = Sd4[2j]   (contiguous, f32)
                Su4o = mpool.tile([128, hw], F32, tag="su4o")
                Sd4e = mpool.tile([128, hw], F32, tag="sd4e")
                for h in range(2):
                    nc.scalar.copy(Su4o[:, h * qw : (h + 1) * qw], SuP[h][:])
                    nc.scalar.copy(Sd4e[:, h * qw : (h + 1) * qw], SdP[h][:])

                Rt = opool.tile([128, 2 * w], BF16, tag="r")
                Gt = opool.tile([128, 2 * w], BF16, tag="g")
                Bt = opool.tile([128, 2 * w], BF16, tag="b")

                # ---- G channel ----
                # (e,e) and (o,o) straight from PSUM (strided bf16 writes)
                for h in range(2):
                    nc.scalar.copy(Gt[:, 2 * h * qw : 2 * (h + 1) * qw : 2], GeP[h][:])
                    nc.scalar.copy(
                        Gt[:, w + 2 * h * qw + 1 : w + 2 * (h + 1) * qw : 2], GoP[h][:]
                    )
                # (e,o): passthrough E odd cols; (o,e): passthrough O even cols
                nc.vector.tensor_copy(Gt[:, 1:w:2], E[:, 1:w:2])
                nc.vector.tensor_copy(Gt[:, w : 2 * w : 2], O[:, 0:w:2])
                nc.sync.dma_start(out[1, 128 * k : 128 * k + 128, :], Gt[:])

                # ---- R channel ----
                # (e,e): passthrough E even cols
                nc.vector.tensor_copy(Rt[:, 0:w:2], E[:, 0:w:2])
                # (e,o): 0.5*(E[x-1] + E[x+1]); te padded to hw, col w-1 fixed after
                te = tpool.tile([128, hw], F32, tag="te")
                nc.vector.tensor_add(te[:, 0 : hw - 1], E[:, 0 : w - 2 : 2], E[:, 2:w:2])
                nc.vector.tensor_copy(te[:, hw - 1 : hw], E[:, w - 2 : w - 1])
                nc.vector.tensor_scalar_mul(Rt[:, 1:w:2], te[:], 0.5)
                # (o,e): 2*Sd4 at even cols
                nc.scalar.mul(Rt[:, w : 2 * w : 2], Sd4e[:], 2.0)
                # (o,o): Sd4[x-1] + Sd4[x+1] = Sd4e[j] + Sd4e[j+1]; last col copy
                nc.vector.tensor_add(
                    Rt[:, w + 1 : 2 * w - 2 : 2], Sd4e[:, 0 : hw - 1], Sd4e[:, 1:hw]
                )
                nc.vector.tensor_copy(
                    Rt[:, 2 * w - 1 : 2 * w], Sd4e[:, hw - 1 : hw]
                )
                nc.sync.dma_start(out[0, 128 * k : 128 * k + 128, :], Rt[:])

                # ---- B channel ----
                # (e,e): Su4[x-1] + Su4[x+1] = Su4o[j-1] + Su4o[j]; col 0 copy
                nc.vector.tensor_add(
                    Bt[:, 2 : w - 1 : 2], Su4o[:, 0 : hw - 1], Su4o[:, 1:hw]
                )
                nc.vector.tensor_copy(Bt[:, 0:1], Su4o[:, 0:1])
                # (e,o): 2*Su4 at odd cols
                nc.vector.tensor_scalar_mul(Bt[:, 1:w:2], Su4o[:], 2.0)
                # (o,e): 0.5*(O[x-1] + O[x+1]); tb padded, col w... col 0 fixed after
                tb = tpool.tile([128, hw], F32, tag="tb")
                nc.vector.tensor_add(tb[:, 1:hw], O[:, 1 : w - 2 : 2], O[:, 3:w:2])
                nc.vector.tensor_copy(tb[:, 0:1], O[:, 1:2])
                nc.scalar.mul(Bt[:, w : 2 * w : 2], tb[:], 0.5)
                # (o,o): passthrough O odd cols
                nc.vector.tensor_copy(Bt[:, w + 1 : 2 * w : 2], O[:, 1:w:2])
                nc.sync.dma_start(out[2, 128 * k : 128 * k + 128, :], Bt[:])

                if k - 1 in INs:
                    del INs[k - 1]

    split_sync_waits(nc)
    return nc


_CACHE = {}


def _get_program(npairs, w):
    key = (npairs, w)
    if key not in _CACHE:
        _CACHE[key] = build_program(npairs, w)
    return _CACHE[key]


def kernel(x: np.ndarray) -> np.ndarray:
    n, _, h, w = x.shape
    assert (n, h, w) == (N_CORES, H, W), x.shape
    nc = _get_program(H // 2, W)
    cmm, cfx = const_arrays()
    in_maps = []
    for i in range(N_CORES):
        img = np.ascontiguousarray(x[i, 0], dtype=np.float32).reshape(H // 2, 2 * W)
        in_maps.append({"x": img, "cmm": cmm, "cfx": cfx})
    res = run_bass_kernel_spmd(nc, in_maps, core_ids=list(range(N_CORES)))
    outs = [
        np.asarray(res.results[i]["out"]).astype(np.float32).reshape(3, H, W)[None]
        for i in range(N_CORES)
    ]
    return np.concatenate(outs, axis=0)


# revision 17
# speedup vs baseline: 4.7148x; 1.0436x over previous
"""Bilinear RGGB demosaic (Bayer -> RGB) on 8 Trainium2 NeuronCores.

Layout: batch image n -> core n. Per core, the [2048, 2048] mosaic is
processed in 8 bands of exactly 128 row-pairs; SBUF partition p of a
band holds the row pair (2p, 2p+1) concatenated in the free dim, so
every DRAM transfer is 16 KiB-contiguous per partition.

Vertical stencil taps are banded 128x128 matmuls (fp32r) on the tensor
engine; band-seam rows are fixed up with K=1 matmuls against the
neighbor band's input tile (accumulated into the same PSUM bank, so no
halo DMA and no overlap bands). The G channel is computed ENTIRELY on
the tensor engine: its horizontal taps are expressed as additional
accumulating matmuls whose moving operands are column-shifted APs of
the same input tile (PSUM accumulation = free adds). Su4/Sd4 are only
produced at the column parity their R/B consumers need, which also
makes every remaining DVE add contiguous.

Outputs are written bf16 (tolerance 2e-2; bf16 rounds at ~2e-3),
halving store traffic; the host widens to f32. Input DMAs ride the SP
HWDGE ring (nc.sync); output DMAs are issued by the otherwise-idle
GpSimd SWDGE so neither compute engine's instruction stream ever
blocks on a store. Elementwise work is balanced across DVE and ACT.
"""

import sys

sys.path.insert(0, "/opt/trn_rl_repo")

import numpy as np

import concourse.bass as bass
import concourse.tile as tile
from concourse import mybir
from concourse.alu_op_type import AluOpType
from concourse.bass_utils import run_bass_kernel_spmd

F32 = mybir.dt.float32
BF16 = mybir.dt.bfloat16
N_CORES = 8
H = 2048
W = 2048
NBANDS = H // 256  # 8 bands of 128 row-pairs


def split_sync_waits(nc, max_waits=1):
    """This walrus build rejects instructions carrying more than
    `max_waits` sync-wait commands. Hoist excess waits onto same-engine
    NoOps inserted immediately before the over-subscribed instruction
    (waiting earlier on the same queue is semantically conservative)."""
    for fn in nc.m.functions:
        for bb in fn.blocks:
            insts = bb.instructions
            i = 0
            while i < len(insts):
                inst = insts[i]
                si = inst.sync_info
                waits = list(si.on_wait) if si and si.on_wait else []
                if len(waits) > max_waits:
                    si.on_wait = waits[:max_waits]
                    excess = waits[max_waits:]
                    for j in range(0, len(excess), max_waits):
                        nop = mybir.InstNoOp(
                            name=nc.get_next_instruction_name(), ins=[], outs=[]
                        )
                        nop.engine = inst.engine
                        nop.sync_info = mybir.SyncInfo(
                            on_wait=excess[j : j + max_waits], on_update=[]
                        )
                        nc.register_instruction(nop)
                        insts.insert(i, nop)
                        i += 1
                i += 1


def const_arrays():
    # cmm[:, 0:128]   m1 : Su[p] = 0.25*(O[p-1] + O[p])
    # cmm[:, 128:256] m2 : Sd[p] = 0.25*(E[p] + E[p+1])
    # cmm[:, 256:384] qI : 0.25 * I (horizontal quarter taps)
    import ml_dtypes

    m1 = 0.25 * (np.eye(128, dtype=np.float32) + np.eye(128, k=1, dtype=np.float32))
    m2 = 0.25 * (np.eye(128, dtype=np.float32) + np.eye(128, k=-1, dtype=np.float32))
    qI = 0.25 * np.eye(128, dtype=np.float32)
    cmm = np.concatenate([m1, m2, qI], axis=1).astype(ml_dtypes.bfloat16)  # [128, 384]
    # cfx[0, 0:128]   fu : row vector, 0.25 into partition 0   (+= 0.25*O_prev)
    # cfx[0, 128:256] fd : row vector, 0.25 into partition 127 (+= 0.25*E_next)
    cfx = np.zeros((1, 256), dtype=np.float32)
    cfx[0, 0] = 0.25
    cfx[0, 128 + 127] = 0.25
    return cmm, cfx.astype(ml_dtypes.bfloat16)


def build_program(npairs=H // 2, w=W):
    nc = bass.Bass("TRN2", target_bir_lowering=False, debug=False)
    x = nc.dram_tensor("x", [npairs, 2 * w], F32, kind="ExternalInput").ap()
    cmm = nc.dram_tensor("cmm", [128, 384], BF16, kind="ExternalInput").ap()
    cfx = nc.dram_tensor("cfx", [1, 256], BF16, kind="ExternalInput").ap()
    out = nc.dram_tensor("out", [3, npairs, 2 * w], BF16, kind="ExternalOutput").ap()

    hw = w // 2  # 1024: outputs of one parity
    qw = w // 4  # 512: one PSUM bank / one half-band of one parity
    nb = npairs // 128
    assert nb * 128 == npairs

    with tile.TileContext(nc) as tc:
        with (
            tc.tile_pool(name="consts", bufs=1) as cpool,
            tc.tile_pool(name="inp", bufs=3) as ipool,
            tc.tile_pool(name="psum", bufs=1, space="PSUM") as ppool,
            tc.tile_pool(name="mids", bufs=2) as mpool,
            tc.tile_pool(name="tmps", bufs=2) as tpool,
            tc.tile_pool(name="outs", bufs=3) as opool,
        ):
            cM = cpool.tile([128, 384], BF16)
            nc.sync.dma_start(cM[:], cmm[:])
            cF = cpool.tile([1, 256], BF16)
            nc.sync.dma_start(cF[:], cfx[:])

            mm = lambda ap: ap
            m1 = mm(cM[:, 0:128])
            m2 = mm(cM[:, 128:256])
            qI = mm(cM[:, 256:384])
            fu = mm(cF[:, 0:128])
            fd = mm(cF[:, 128:256])

            INs = {}

            def get_in(k):
                # two DMAs: E half lands first so Sd/R work starts earlier
                if k not in INs:
                    t = ipool.tile([128, 2 * w], BF16, tag="in")
                    r = slice(128 * k, 128 * k + 128)
                    nc.gpsimd.dma_start(t[:, 0:w], x[r, 0:w])
                    nc.gpsimd.dma_start(t[:, w : 2 * w], x[r, w : 2 * w])
                    INs[k] = t
                return INs[k]

            for k in range(nb):
                IN = get_in(k)
                INn = get_in(k + 1) if k + 1 < nb else None

                E = IN[:, 0:w]
                O = IN[:, w : 2 * w]
                # halo rows, single partition each. The next band's first E
                # row is partition 0 of its input tile (a legal matmul base
                # partition); the previous band's last O row would sit at
                # partition 127, so re-fetch those 8 KiB into partition 0.
                Op = None
                if k > 0:
                    OpH = tpool.tile([1, w], BF16, tag="oph")
                    nc.gpsimd.dma_start(OpH[:], x[128 * k - 1 : 128 * k, w : 2 * w])
                    Op = OpH[:]
                En = INn[0:1, 0:w] if INn is not None else None

                # PSUM, per half h (columns [1024h, 1024h+1024)):
                #   SuP[h] [128,512]: Su4 at odd cols of the half
                #   SdP[h] [128,512]: Su4 at even cols
                #   GeP[h] [128,512]: G at (even row, even col)
                #   GoP[h] [128,512]: G at (odd row, odd col)
                SuP0 = ppool.tile([128, qw], F32, tag="su0")
                SuP1 = ppool.tile([128, qw], F32, tag="su1")
                SdP0 = ppool.tile([128, qw], F32, tag="sd0")
                SdP1 = ppool.tile([128, qw], F32, tag="sd1")
                GeP0 = ppool.tile([128, qw], F32, tag="ge0")
                GeP1 = ppool.tile([128, qw], F32, tag="ge1")
                GoP0 = ppool.tile([128, qw], F32, tag="go0")
                GoP1 = ppool.tile([128, qw], F32, tag="go1")
                SuP = [SuP0, SuP1]
                SdP = [SdP0, SdP1]
                GeP = [GeP0, GeP1]
                GoP = [GoP0, GoP1]

                MM = nc.tensor.matmul
                rows = slice(128 * k, 128 * k + 128)

                # ---- Sd matmuls (need only the E half of the input) ----
                for h in range(2):
                    c = hw * h
                    MM(SdP[h][:], m2, mm(E[:, c : c + hw : 2]),
                       start=True, stop=(INn is None))
                for h in range(2):
                    c = hw * h
                    if INn is not None:
                        MM(SdP[h][:], fd, mm(En[:, c : c + hw : 2]),
                           start=False, stop=True, skip_group_check=True)

                # Sd4e[j] = Sd4[2j] (contiguous, f32)
                Sd4e = mpool.tile([128, hw], F32, tag="sd4e")
                for h in range(2):
                    nc.scalar.copy(Sd4e[:, h * qw : (h + 1) * qw], SdP[h][:])

                Rt = opool.tile([128, 2 * w], BF16, tag="r")
                Gt = opool.tile([128, 2 * w], BF16, tag="g")
                Bt = opool.tile([128, 2 * w], BF16, tag="b")

                # ---- R channel (E + Sd4e only) ----
                # (e,e): passthrough E even cols
                nc.vector.tensor_copy(Rt[:, 0:w:2], E[:, 0:w:2])
                # (e,o): 0.5*(E[x-1] + E[x+1]); te padded to hw, col w-1 via pad
                te = tpool.tile([128, hw], F32, tag="te")
                nc.vector.tensor_add(te[:, 0 : hw - 1], E[:, 0 : w - 2 : 2], E[:, 2:w:2])
                nc.vector.tensor_copy(te[:, hw - 1 : hw], E[:, w - 2 : w - 1])
                nc.vector.tensor_scalar_mul(Rt[:, 1:w:2], te[:], 0.5)
                # (o,e): 2*Sd4 at even cols
                nc.scalar.mul(Rt[:, w : 2 * w : 2], Sd4e[:], 2.0)
                # (o,o): Sd4[x-1] + Sd4[x+1] = Sd4e[j] + Sd4e[j+1]; last col copy
                nc.vector.tensor_add(
                    Rt[:, w + 1 : 2 * w - 2 : 2], Sd4e[:, 0 : hw - 1], Sd4e[:, 1:hw]
                )
                nc.vector.tensor_copy(
                    Rt[:, 2 * w - 1 : 2 * w], Sd4e[:, hw - 1 : hw]
                )
                nc.sync.dma_start(out[0, rows, 0:w], Rt[:, 0:w])
                nc.sync.dma_start(out[0, rows, w : 2 * w], Rt[:, w : 2 * w])

                # ---- Su matmuls (O half) ----
                for h in range(2):
                    c = hw * h
                    MM(SuP[h][:], m1, mm(O[:, c + 1 : c + hw : 2]),
                       start=True, stop=(Op is None))
                for h in range(2):
                    c = hw * h
                    if Op is not None:
                        MM(SuP[h][:], fu, mm(Op[:, c + 1 : c + hw : 2]),
                           start=False, stop=True, skip_group_check=True)

                # Su4o[j] = Su4[2j+1] (contiguous, f32)
                Su4o = mpool.tile([128, hw], F32, tag="su4o")
                for h in range(2):
                    nc.scalar.copy(Su4o[:, h * qw : (h + 1) * qw], SuP[h][:])

                # ---- B channel (O + Su4o only) ----
                # (e,e): Su4[x-1] + Su4[x+1] = Su4o[j-1] + Su4o[j]; col 0 copy
                nc.vector.tensor_add(
                    Bt[:, 2 : w - 1 : 2], Su4o[:, 0 : hw - 1], Su4o[:, 1:hw]
                )
                nc.vector.tensor_copy(Bt[:, 0:1], Su4o[:, 0:1])
                # (e,o): 2*Su4 at odd cols
                nc.vector.tensor_scalar_mul(Bt[:, 1:w:2], Su4o[:], 2.0)
                # (o,e): 0.5*(O[x-1] + O[x+1]); col 0 via tb pad
                tb = tpool.tile([128, hw], F32, tag="tb")
                nc.vector.tensor_add(tb[:, 1:hw], O[:, 1 : w - 2 : 2], O[:, 3:w:2])
                nc.vector.tensor_copy(tb[:, 0:1], O[:, 1:2])
                nc.scalar.mul(Bt[:, w : 2 * w : 2], tb[:], 0.5)
                # (o,o): passthrough O odd cols
                nc.vector.tensor_copy(Bt[:, w + 1 : 2 * w : 2], O[:, 1:w:2])
                nc.sync.dma_start(out[2, rows, 0:w], Bt[:, 0:w])
                nc.sync.dma_start(out[2, rows, w : 2 * w], Bt[:, w : 2 * w])

                # ---- Ge matmuls: G(e,e) = Su4@even + 0.25*(E[x-1]+E[x+1]) ----
                for h in range(2):
                    c = hw * h
                    MM(GeP[h][:], m1, mm(O[:, c : c + hw : 2]),
                       start=True, stop=False)
                for h in range(2):
                    c = hw * h
                    if c == 0:
                        MM(GeP[h][:, 1:qw], qI, mm(E[:, 1 : hw - 1 : 2]),
                           start=False, stop=False, skip_group_check=True)
                    else:
                        MM(GeP[h][:], qI, mm(E[:, c - 1 : c + hw - 1 : 2]),
                           start=False, stop=False, skip_group_check=True)
                    MM(GeP[h][:], qI, mm(E[:, c + 1 : min(c + hw + 1, w) : 2]),
                       start=False, stop=(Op is None), skip_group_check=True)
                for h in range(2):
                    c = hw * h
                    if Op is not None:
                        MM(GeP[h][:], fu, mm(Op[:, c : c + hw : 2]),
                           start=False, stop=True, skip_group_check=True)

                # G even rows: (e,e) from PSUM + (e,o) passthrough E odd cols
                for h in range(2):
                    nc.scalar.copy(Gt[:, 2 * h * qw : 2 * (h + 1) * qw : 2], GeP[h][:])
                nc.vector.tensor_copy(Gt[:, 1:w:2], E[:, 1:w:2])
                nc.sync.dma_start(out[1, rows, 0:w], Gt[:, 0:w])

                # ---- Go matmuls: G(o,o) = Sd4@odd + 0.25*(O[x-1]+O[x+1]) ----
                for h in range(2):
                    c = hw * h
                    MM(GoP[h][:], m2, mm(E[:, c + 1 : c + hw : 2]),
                       start=True, stop=False)
                for h in range(2):
                    c = hw * h
                    MM(GoP[h][:], qI, mm(O[:, c : c + hw : 2]),
                       start=False, stop=False, skip_group_check=True)
                    if c + hw == w:
                        MM(GoP[h][:, 0 : qw - 1], qI, mm(O[:, c + 2 : c + hw : 2]),
                           start=False, stop=(INn is None), skip_group_check=True)
                    else:
                        MM(GoP[h][:], qI, mm(O[:, c + 2 : c + hw + 2 : 2]),
                           start=False, stop=(INn is None), skip_group_check=True)
                for h in range(2):
                    c = hw * h
                    if INn is not None:
                        MM(GoP[h][:], fd, mm(En[:, c + 1 : c + hw : 2]),
                           start=False, stop=True, skip_group_check=True)

                # G odd rows: (o,o) from PSUM + (o,e) passthrough O even cols
                for h in range(2):
                    nc.scalar.copy(
                        Gt[:, w + 2 * h * qw + 1 : w + 2 * (h + 1) * qw : 2], GoP[h][:]
                    )
                nc.vector.tensor_copy(Gt[:, w : 2 * w : 2], O[:, 0:w:2])
                nc.sync.dma_start(out[1, rows, w : 2 * w], Gt[:, w : 2 * w])

                if k - 1 in INs:
                    del INs[k - 1]

    split_sync_waits(nc)
    return nc


_CACHE = {}


def _get_program(npairs, w):
    key = (npairs, w)
    if key not in _CACHE:
        _CACHE[key] = build_program(npairs, w)
    return _CACHE[key]


def kernel(x: np.ndarray) -> np.ndarray:
    n, _, h, w = x.shape
    assert (n, h, w) == (N_CORES, H, W), x.shape
    nc = _get_program(H // 2, W)
    cmm, cfx = const_arrays()
    in_maps = []
    for i in range(N_CORES):
        img = np.ascontiguousarray(x[i, 0], dtype=np.float32).reshape(H // 2, 2 * W)
        in_maps.append({"x": img, "cmm": cmm, "cfx": cfx})
    res = run_bass_kernel_spmd(nc, in_maps, core_ids=list(range(N_CORES)))
    outs = [
        np.asarray(res.results[i]["out"]).astype(np.float32).reshape(3, H, W)[None]
        for i in range(N_CORES)
    ]
    return np.concatenate(outs, axis=0)


# revision 18
# speedup vs baseline: 5.0769x; 1.0768x over previous
"""Bilinear RGGB demosaic (Bayer -> RGB) on 8 Trainium2 NeuronCores.

Layout: batch image n -> core n. Per core, the [2048, 2048] mosaic is
processed in 8 bands of exactly 128 row-pairs; SBUF partition p of a
band holds the row pair (2p, 2p+1) concatenated in the free dim, so
every DRAM transfer is 16 KiB-contiguous per partition.

Vertical stencil taps are banded 128x128 matmuls (fp32r) on the tensor
engine; band-seam rows are fixed up with K=1 matmuls against the
neighbor band's input tile (accumulated into the same PSUM bank, so no
halo DMA and no overlap bands). The G channel is computed ENTIRELY on
the tensor engine: its horizontal taps are expressed as additional
accumulating matmuls whose moving operands are column-shifted APs of
the same input tile (PSUM accumulation = free adds). Su4/Sd4 are only
produced at the column parity their R/B consumers need, which also
makes every remaining DVE add contiguous.

Outputs are written bf16 (tolerance 2e-2; bf16 rounds at ~2e-3),
halving store traffic; the host widens to f32. Input DMAs ride the SP
HWDGE ring (nc.sync); output DMAs are issued by the otherwise-idle
GpSimd SWDGE so neither compute engine's instruction stream ever
blocks on a store. Elementwise work is balanced across DVE and ACT.
"""

import sys

sys.path.insert(0, "/opt/trn_rl_repo")

import numpy as np

import concourse.bass as bass
import concourse.tile as tile
from concourse import mybir
from concourse.alu_op_type import AluOpType
from concourse.bass_utils import run_bass_kernel_spmd

F32 = mybir.dt.float32
BF16 = mybir.dt.bfloat16
N_CORES = 8
H = 2048
W = 2048
NBANDS = H // 256  # 8 bands of 128 row-pairs


def split_sync_waits(nc, max_waits=1):
    """This walrus build rejects instructions carrying more than
    `max_waits` sync-wait commands. Hoist excess waits onto same-engine
    NoOps inserted immediately before the over-subscribed instruction
    (waiting earlier on the same queue is semantically conservative)."""
    for fn in nc.m.functions:
        for bb in fn.blocks:
            insts = bb.instructions
            i = 0
            while i < len(insts):
                inst = insts[i]
                si = inst.sync_info
                waits = list(si.on_wait) if si and si.on_wait else []
                if len(waits) > max_waits:
                    si.on_wait = waits[:max_waits]
                    excess = waits[max_waits:]
                    for j in range(0, len(excess), max_waits):
                        nop = mybir.InstNoOp(
                            name=nc.get_next_instruction_name(), ins=[], outs=[]
                        )
                        nop.engine = inst.engine
                        nop.sync_info = mybir.SyncInfo(
                            on_wait=excess[j : j + max_waits], on_update=[]
                        )
                        nc.register_instruction(nop)
                        insts.insert(i, nop)
                        i += 1
                i += 1


def const_arrays():
    # cmm[:, 0:128]   m1 : Su[p] = 0.25*(O[p-1] + O[p])
    # cmm[:, 128:256] m2 : Sd[p] = 0.25*(E[p] + E[p+1])
    # cmm[:, 256:384] qI : 0.25 * I (horizontal quarter taps)
    import ml_dtypes

    m1 = 0.25 * (np.eye(128, dtype=np.float32) + np.eye(128, k=1, dtype=np.float32))
    m2 = 0.25 * (np.eye(128, dtype=np.float32) + np.eye(128, k=-1, dtype=np.float32))
    qI = 0.25 * np.eye(128, dtype=np.float32)
    cmm = np.concatenate([m1, m2, qI], axis=1).astype(ml_dtypes.bfloat16)  # [128, 384]
    # cfx[0, 0:128]   fu : row vector, 0.25 into partition 0   (+= 0.25*O_prev)
    # cfx[0, 128:256] fd : row vector, 0.25 into partition 127 (+= 0.25*E_next)
    cfx = np.zeros((1, 256), dtype=np.float32)
    cfx[0, 0] = 0.25
    cfx[0, 128 + 127] = 0.25
    return cmm, cfx.astype(ml_dtypes.bfloat16)


def build_program(npairs=H // 2, w=W):
    nc = bass.Bass("TRN2", target_bir_lowering=False, debug=False)
    x = nc.dram_tensor("x", [npairs, 2 * w], F32, kind="ExternalInput").ap()
    cmm = nc.dram_tensor("cmm", [128, 384], BF16, kind="ExternalInput").ap()
    cfx = nc.dram_tensor("cfx", [1, 256], BF16, kind="ExternalInput").ap()
    out = nc.dram_tensor("out", [3, npairs, 2 * w], BF16, kind="ExternalOutput").ap()

    hw = w // 2  # 1024: outputs of one parity
    qw = w // 4  # 512: one PSUM bank / one half-band of one parity
    nb = npairs // 128
    assert nb * 128 == npairs

    with tile.TileContext(nc) as tc:
        with (
            tc.tile_pool(name="consts", bufs=1) as cpool,
            tc.tile_pool(name="inp", bufs=4) as ipool,
            tc.tile_pool(name="psum", bufs=1, space="PSUM") as ppool,
            tc.tile_pool(name="mids", bufs=2) as mpool,
            tc.tile_pool(name="tmps", bufs=2) as tpool,
            tc.tile_pool(name="outs", bufs=3) as opool,
        ):
            cM = cpool.tile([128, 384], BF16)
            nc.sync.dma_start(cM[:], cmm[:])
            cF = cpool.tile([1, 256], BF16)
            nc.sync.dma_start(cF[:], cfx[:])

            mm = lambda ap: ap
            m1 = mm(cM[:, 0:128])
            m2 = mm(cM[:, 128:256])
            qI = mm(cM[:, 256:384])
            fu = mm(cF[:, 0:128])
            fd = mm(cF[:, 128:256])

            INs = {}

            def get_in(k):
                # two DMAs: E half lands first so Sd/R work starts earlier
                if k not in INs:
                    t = ipool.tile([128, 2 * w], BF16, tag="in")
                    r = slice(128 * k, 128 * k + 128)
                    nc.gpsimd.dma_start(t[:, 0:w], x[r, 0:w])
                    nc.gpsimd.dma_start(t[:, w : 2 * w], x[r, w : 2 * w])
                    INs[k] = t
                return INs[k]

            for k in range(nb):
                IN = get_in(k)
                INn = get_in(k + 1) if k + 1 < nb else None
                if k + 2 < nb:
                    get_in(k + 2)  # deeper prefetch keeps HBM reads ahead

                E = IN[:, 0:w]
                O = IN[:, w : 2 * w]
                # halo rows, single partition each. The next band's first E
                # row is partition 0 of its input tile (a legal matmul base
                # partition); the previous band's last O row would sit at
                # partition 127, so re-fetch those 8 KiB into partition 0.
                Op = None
                if k > 0:
                    OpH = tpool.tile([1, w], BF16, tag="oph")
                    nc.gpsimd.dma_start(OpH[:], x[128 * k - 1 : 128 * k, w : 2 * w])
                    Op = OpH[:]
                En = INn[0:1, 0:w] if INn is not None else None

                # PSUM, per half h (columns [1024h, 1024h+1024)):
                #   SuP[h] [128,512]: Su4 at odd cols of the half
                #   SdP[h] [128,512]: Su4 at even cols
                #   GeP[h] [128,512]: G at (even row, even col)
                #   GoP[h] [128,512]: G at (odd row, odd col)
                SuP0 = ppool.tile([128, qw], F32, tag="su0")
                SuP1 = ppool.tile([128, qw], F32, tag="su1")
                SdP0 = ppool.tile([128, qw], F32, tag="sd0")
                SdP1 = ppool.tile([128, qw], F32, tag="sd1")
                GeP0 = ppool.tile([128, qw], F32, tag="ge0")
                GeP1 = ppool.tile([128, qw], F32, tag="ge1")
                GoP0 = ppool.tile([128, qw], F32, tag="go0")
                GoP1 = ppool.tile([128, qw], F32, tag="go1")
                SuP = [SuP0, SuP1]
                SdP = [SdP0, SdP1]
                GeP = [GeP0, GeP1]
                GoP = [GoP0, GoP1]

                MM = nc.tensor.matmul
                rows = slice(128 * k, 128 * k + 128)

                # ---- Sd matmuls (need only the E half of the input) ----
                for h in range(2):
                    c = hw * h
                    MM(SdP[h][:], m2, mm(E[:, c : c + hw : 2]),
                       start=True, stop=(INn is None))
                for h in range(2):
                    c = hw * h
                    if INn is not None:
                        MM(SdP[h][:], fd, mm(En[:, c : c + hw : 2]),
                           start=False, stop=True, skip_group_check=True)

                # Sd4e[j] = Sd4[2j] (contiguous, f32)
                Sd4e = mpool.tile([128, hw], F32, tag="sd4e")
                for h in range(2):
                    nc.scalar.copy(Sd4e[:, h * qw : (h + 1) * qw], SdP[h][:])

                Rt = opool.tile([128, 2 * w], BF16, tag="r")
                Gt = opool.tile([128, 2 * w], BF16, tag="g")
                Bt = opool.tile([128, 2 * w], BF16, tag="b")

                # ---- R channel (E + Sd4e only) ----
                # (e,e): passthrough E even cols
                nc.vector.tensor_copy(Rt[:, 0:w:2], E[:, 0:w:2])
                # (e,o): 0.5*(E[x-1] + E[x+1]); te padded to hw, col w-1 via pad
                te = tpool.tile([128, hw], F32, tag="te")
                nc.vector.tensor_add(te[:, 0 : hw - 1], E[:, 0 : w - 2 : 2], E[:, 2:w:2])
                nc.vector.tensor_copy(te[:, hw - 1 : hw], E[:, w - 2 : w - 1])
                nc.vector.tensor_scalar_mul(Rt[:, 1:w:2], te[:], 0.5)
                # (o,e): 2*Sd4 at even cols
                nc.scalar.mul(Rt[:, w : 2 * w : 2], Sd4e[:], 2.0)
                # (o,o): Sd4[x-1] + Sd4[x+1] = Sd4e[j] + Sd4e[j+1]; last col copy
                nc.vector.tensor_add(
                    Rt[:, w + 1 : 2 * w - 2 : 2], Sd4e[:, 0 : hw - 1], Sd4e[:, 1:hw]
                )
                nc.vector.tensor_copy(
                    Rt[:, 2 * w - 1 : 2 * w], Sd4e[:, hw - 1 : hw]
                )
                nc.sync.dma_start(out[0, rows, 0:w], Rt[:, 0:w])
                nc.sync.dma_start(out[0, rows, w : 2 * w], Rt[:, w : 2 * w])

                # ---- Su matmuls (O half) ----
                for h in range(2):
                    c = hw * h
                    MM(SuP[h][:], m1, mm(O[:, c + 1 : c + hw : 2]),
                       start=True, stop=(Op is None))
                for h in range(2):
                    c = hw * h
                    if Op is not None:
                        MM(SuP[h][:], fu, mm(Op[:, c + 1 : c + hw : 2]),
                           start=False, stop=True, skip_group_check=True)

                # Su4o[j] = Su4[2j+1] (contiguous, f32)
                Su4o = mpool.tile([128, hw], F32, tag="su4o")
                for h in range(2):
                    nc.scalar.copy(Su4o[:, h * qw : (h + 1) * qw], SuP[h][:])

                # ---- B channel (O + Su4o only) ----
                # (e,e): Su4[x-1] + Su4[x+1] = Su4o[j-1] + Su4o[j]; col 0 copy
                nc.vector.tensor_add(
                    Bt[:, 2 : w - 1 : 2], Su4o[:, 0 : hw - 1], Su4o[:, 1:hw]
                )
                nc.vector.tensor_copy(Bt[:, 0:1], Su4o[:, 0:1])
                # (e,o): 2*Su4 at odd cols
                nc.vector.tensor_scalar_mul(Bt[:, 1:w:2], Su4o[:], 2.0)
                # (o,e): 0.5*(O[x-1] + O[x+1]); col 0 via tb pad
                tb = tpool.tile([128, hw], F32, tag="tb")
                nc.vector.tensor_add(tb[:, 1:hw], O[:, 1 : w - 2 : 2], O[:, 3:w:2])
                nc.vector.tensor_copy(tb[:, 0:1], O[:, 1:2])
                nc.scalar.mul(Bt[:, w : 2 * w : 2], tb[:], 0.5)
                # (o,o): passthrough O odd cols
                nc.vector.tensor_copy(Bt[:, w + 1 : 2 * w : 2], O[:, 1:w:2])
                nc.sync.dma_start(out[2, rows, 0:w], Bt[:, 0:w])
                nc.sync.dma_start(out[2, rows, w : 2 * w], Bt[:, w : 2 * w])

                # ---- Ge matmuls: G(e,e) = Su4@even + 0.25*(E[x-1]+E[x+1]) ----
                for h in range(2):
                    c = hw * h
                    MM(GeP[h][:], m1, mm(O[:, c : c + hw : 2]),
                       start=True, stop=False)
                for h in range(2):
                    c = hw * h
                    if c == 0:
                        MM(GeP[h][:, 1:qw], qI, mm(E[:, 1 : hw - 1 : 2]),
                           start=False, stop=False, skip_group_check=True)
                    else:
                        MM(GeP[h][:], qI, mm(E[:, c - 1 : c + hw - 1 : 2]),
                           start=False, stop=False, skip_group_check=True)
                    MM(GeP[h][:], qI, mm(E[:, c + 1 : min(c + hw + 1, w) : 2]),
                       start=False, stop=(Op is None), skip_group_check=True)
                for h in range(2):
                    c = hw * h
                    if Op is not None:
                        MM(GeP[h][:], fu, mm(Op[:, c : c + hw : 2]),
                           start=False, stop=True, skip_group_check=True)

                # G even rows: (e,e) from PSUM + (e,o) passthrough E odd cols
                for h in range(2):
                    nc.scalar.copy(Gt[:, 2 * h * qw : 2 * (h + 1) * qw : 2], GeP[h][:])
                nc.vector.tensor_copy(Gt[:, 1:w:2], E[:, 1:w:2])
                nc.sync.dma_start(out[1, rows, 0:w], Gt[:, 0:w])

                # ---- Go matmuls: G(o,o) = Sd4@odd + 0.25*(O[x-1]+O[x+1]) ----
                for h in range(2):
                    c = hw * h
                    MM(GoP[h][:], m2, mm(E[:, c + 1 : c + hw : 2]),
                       start=True, stop=False)
                for h in range(2):
                    c = hw * h
                    MM(GoP[h][:], qI, mm(O[:, c : c + hw : 2]),
                       start=False, stop=False, skip_group_check=True)
                    if c + hw == w:
                        MM(GoP[h][:, 0 : qw - 1], qI, mm(O[:, c + 2 : c + hw : 2]),
                           start=False, stop=(INn is None), skip_group_check=True)
                    else:
                        MM(GoP[h][:], qI, mm(O[:, c + 2 : c + hw + 2 : 2]),
                           start=False, stop=(INn is None), skip_group_check=True)
                for h in range(2):
                    c = hw * h
                    if INn is not None:
                        MM(GoP[h][:], fd, mm(En[:, c + 1 : c + hw : 2]),
                           start=False, stop=True, skip_group_check=True)

                # G odd rows: (o,o) from PSUM + (o,e) passthrough O even cols
                for h in range(2):
                    nc.scalar.copy(
                        Gt[:, w + 2 * h * qw + 1 : w + 2 * (h + 1) * qw : 2], GoP[h][:]
                    )
                nc.vector.tensor_copy(Gt[:, w : 2 * w : 2], O[:, 0:w:2])
                nc.sync.dma_start(out[1, rows, w : 2 * w], Gt[:, w : 2 * w])

                if k - 1 in INs:
                    del INs[k - 1]

    split_sync_waits(nc)
    return nc


_CACHE = {}


def _get_program(npairs, w):
    key = (npairs, w)
    if key not in _CACHE:
        _CACHE[key] = build_program(npairs, w)
    return _CACHE[key]


def kernel(x: np.ndarray) -> np.ndarray:
    n, _, h, w = x.shape
    assert (n, h, w) == (N_CORES, H, W), x.shape
    nc = _get_program(H // 2, W)
    cmm, cfx = const_arrays()
    in_maps = []
    for i in range(N_CORES):
        img = np.ascontiguousarray(x[i, 0], dtype=np.float32).reshape(H // 2, 2 * W)
        in_maps.append({"x": img, "cmm": cmm, "cfx": cfx})
    res = run_bass_kernel_spmd(nc, in_maps, core_ids=list(range(N_CORES)))
    outs = [
        np.asarray(res.results[i]["out"]).astype(np.float32).reshape(3, H, W)[None]
        for i in range(N_CORES)
    ]
    return np.concatenate(outs, axis=0)
